# revision 32
# baseline (speedup 1.0000x reference)
"""Multi-head self-attention (B=4, S=2048, D=1024, H=16) on 8 NeuronCores.

Sharding: tensor-parallel over heads. Core c owns heads {2c, 2c+1} = 128
columns of Wq/Wk/Wv and 128 rows of Wo. Each core computes Q^T/K^T/V for its
two heads over all tokens, runs attention for its 8 (batch, head) pairs, and
produces a partial output O_c = A_c @ Wo_c. The all-reduce over the 8
partials is done on the host during unsharding.

v2: software-pipelined at key-chunk granularity so the tensor engine never
stalls on the softmax exp (which would drop it from its 2.4 GHz ramped
p-state back to 1.2 GHz). The scalar engine runs only the exp activations
(the true bottleneck at ~285 us); QKV projections for the next batch,
V transposes, and the output projection of the previous q-tile are emitted
as tensor-engine fillers between score/AV matmuls. Softmax normalization is
restructured: denominators (from a ones-column in the V stationary) are
transposed into partition-parallel layout with tiny stationary-[1,128]
matmuls, reciprocated as [128,8] on the DVE (203ns vs 3.3us for [1,512]),
transposed back via the PE, and applied in a fused normalize+evacuate
tensor_tensor multiply.
"""
import os
import sys

for _p in ("/opt/trn_rl_repo", "/root/.axon_site/_ro/trn_rl_repo"):
    if os.path.isdir(_p) and _p not in sys.path:
        sys.path.append(_p)

from collections import deque
from contextlib import ExitStack

import numpy as np
import ml_dtypes

import concourse.bass as bass
import concourse.tile as tile
from concourse import mybir
from concourse.bass_utils import run_bass_kernel_spmd
from concourse.masks import make_identity

BF16 = mybir.dt.bfloat16
F32 = mybir.dt.float32
EXP = mybir.ActivationFunctionType.Exp
NP_BF16 = ml_dtypes.bfloat16

B, S, D = 4, 2048, 1024
H, HD = 16, 64
N_CORES = 8
T = B * S  # 8192 tokens
KC = D // 128  # 8 contraction chunks
NKC = S // 128  # 16 key chunks per batch
SCALE = 1.0 / np.sqrt(HD)
LAG = 5  # AV stream lags the score stream by this many kc-steps

# ---------------------------------------------------------------------------
# Tile patches: this walrus build rejects instructions with more than one
# sync wait ("Too many sync wait commands"), so split extra waits into
# preceding same-engine nops, and replace the kernel-tail drain's wait list
# with a chain of single-wait SP nops.
# ---------------------------------------------------------------------------
_MAX_WAITS = 1
_patched = False


def _install_tile_patches():
    global _patched
    if _patched:
        return
    _patched = True
    from concourse.vector_clock import ScopedClock, VectorClock

    orig_lower = tile.TileContext._lower_ordered_insts

    def split_inst_waits(self, ordered):
        for bb_name in list(ordered.keys()):
            insts = ordered[bb_name]
            new = []
            for inst in insts:
                si = inst.sync_info
                if si is not None and len(si.on_wait) > _MAX_WAITS:
                    waits = list(si.on_wait)
                    head, tail = waits[:-_MAX_WAITS], waits[-_MAX_WAITS:]
                    for w in head:
                        nop = mybir.InstNoOp(
                            name=f"ws-{self.nc.next_id()}",
                            engine=inst.engine,
                            bass_nofuse=True,
                        )
                        nop.sync_info = mybir.SyncInfo(on_wait=[w], on_update=[])
                        new.append(nop)
                    inst.sync_info = mybir.SyncInfo(
                        on_wait=tail, on_update=list(si.on_update)
                    )
                new.append(inst)
            ordered[bb_name] = new
        return orig_lower(self, ordered)

    def split_drain_and_barrier(self, tick_clock, wait_clock):
        gc = tick_clock.global_clock
        ticks = eval(repr(gc).replace("VectorClock", ""))
        procs = [(i, t) for i, t in enumerate(ticks) if t > 0]
        for i in range(0, len(procs), _MAX_WAITS):
            chunk = procs[i : i + _MAX_WAITS]
            nop = self.nc.sync.nop(nofuse=True, hint="drain_wait_split")
            pc = VectorClock()
            for proc, tick in chunk:
                pc.require_at_least(proc, tick)
            wait_clock.add_sem_waits(nop.ins, ScopedClock({None: pc}))
        drain_inst = self.nc.sync.drain()
        wait_clock.add_sem_waits(
            drain_inst.ins, ScopedClock({None: gc}), ScopedClock({None: gc.copy()})
        )
        self.nc.all_engine_barrier()
        assert self.sems is not None
        popped = self.nc._tile_sem_poison_stack.pop()
        assert popped is self._sem_poison
        self.nc.clear_and_free_semaphores(list(self.sems.allocated().values()))
        self.nc.all_engine_barrier()

    tile.TileContext._lower_ordered_insts = split_inst_waits
    tile.TileContext._drain_and_barrier = split_drain_and_barrier


# ---------------------------------------------------------------------------
# Device kernel
# ---------------------------------------------------------------------------
def build_attention_nc(with_bias=False):
    _install_tile_patches()
    nc = bass.Bass()

    xT = nc.declare_dram_parameter("xT", [KC, 128, T], BF16, isOutput=False)
    wq = nc.declare_dram_parameter("wq", [KC, 128, 128], BF16, isOutput=False)
    wk = nc.declare_dram_parameter("wk", [KC, 128, 128], BF16, isOutput=False)
    wv = nc.declare_dram_parameter("wv", [KC, 128, 128], BF16, isOutput=False)
    if with_bias:
        bq = nc.declare_dram_parameter("bq", [128], BF16, isOutput=False)
        bk = nc.declare_dram_parameter("bk", [128], BF16, isOutput=False)
        bv = nc.declare_dram_parameter("bv", [128], BF16, isOutput=False)
    else:
        bq = bk = bv = None
    wo = nc.declare_dram_parameter("wo", [128, D], BF16, isOutput=False)
    out = nc.declare_dram_parameter("out", [T, D], BF16, isOutput=True)

    with tile.TileContext(nc) as tc, ExitStack() as ctx:
        singles = ctx.enter_context(tc.tile_pool(name="singles", bufs=1))
        px = ctx.enter_context(tc.tile_pool(name="px", bufs=16))
        pqk = ctx.enter_context(tc.tile_pool(name="pqk", bufs=2))
        pv = ctx.enter_context(tc.tile_pool(name="pv", bufs=2))
        ppt = ctx.enter_context(tc.tile_pool(name="ppt", bufs=LAG + 2))
        pa = ctx.enter_context(tc.tile_pool(name="pa", bufs=3))
        pob = ctx.enter_context(tc.tile_pool(name="pob", bufs=3))
        pd = ctx.enter_context(tc.tile_pool(name="pd", bufs=2))
        pbc = ctx.enter_context(tc.tile_pool(name="pbc", bufs=2))
        dsc = ctx.enter_context(tc.tile_pool(name="dsc", bufs=4, space="DRAM"))
        psS = ctx.enter_context(tc.tile_pool(name="psS", bufs=2, space="PSUM"))
        psU = ctx.enter_context(tc.tile_pool(name="psU", bufs=1, space="PSUM"))
        psQ = ctx.enter_context(tc.tile_pool(name="psQ", bufs=1, space="PSUM"))
        psA = ctx.enter_context(tc.tile_pool(name="psA", bufs=1, space="PSUM"))

        # --- constants / weights, loaded once -----------------------------
        w_sb = {}
        b_sb = {}
        for name, wd, bd in (("q", wq, bq), ("k", wk, bk), ("v", wv, bv)):
            w_t = singles.tile([128, KC, 128], BF16, tag=f"w{name}")
            nc.sync.dma_start(w_t, wd[:, :, :].rearrange("k p m -> p k m"))
            w_sb[name] = w_t
            if with_bias:
                b_t = singles.tile([1, 128], BF16, tag=f"b{name}")
                nc.sync.dma_start(b_t, bd[:][None, :])
                b_sb[name] = b_t
        wo_sb = singles.tile([128, D], BF16, tag="wo")
        nc.sync.dma_start(wo_sb, wo[:, :])
        ones_row = singles.tile([1, 512], BF16, tag="ones_row")
        nc.vector.memset(ones_row, 1.0)
        ones1 = singles.tile([128, 1], BF16, tag="ones1")
        nc.vector.memset(ones1, 1.0)
        ident = singles.tile([128, 128], BF16, tag="ident")
        make_identity(nc, ident)
        identf = singles.tile([128, 128], F32, tag="identf")
        make_identity(nc, identf)

        # u01: single PSUM accumulator [128, 1024] f32; head0 in cols 0:512,
        # head1 in cols 512:1024; row 64 collects softmax denominators via
        # the ones column (index 64 resp. 129) in the vS stationary.
        u01 = psU.tile([128, 1024], F32, tag="u01")

        state = [dict() for _ in range(B)]

        # ------------------------------------------------------------------
        # filler machinery: small closures, each ~1-2 PE instructions (plus
        # attached DVE/DMA work). Popped between score/AV steps.
        # ------------------------------------------------------------------
        fill_q = deque()
        defer_q = deque()

        def pop_fillers_budget(budget_ns):
            # fillers return their PE cost (ns); pop until budget exhausted
            spent = 0
            while fill_q and spent < budget_ns:
                spent += fill_q.popleft()() or 216
            return spent

        def pop_fillers(n):
            for _ in range(n):
                if not fill_q:
                    return
                fill_q.popleft()()

        def drain_fillers():
            while fill_q:
                fill_q.popleft()()

        # --- QKV projection / V transpose fillers for batch b -------------
        def push_x_dmas(b, part=0):
            """part 0: allocate tiles + first token-quarter slices (needed
            first by the QKV fillers); parts 1..3: remaining quarters,
            staggered so the Sync queue is never occupied for long."""
            st = state[b]
            if part == 0:
                st["x"] = [
                    px.tile([128, S], BF16, tag="x", name=f"x_{b}_{kc}")
                    for kc in range(KC)
                ]
            lo, hi = part * 512, (part + 1) * 512
            eng = nc.sync if part < 2 else nc.gpsimd
            for kc in range(KC):
                eng.dma_start(
                    st["x"][kc][:, lo:hi],
                    xT[kc, :, b * S + lo : b * S + hi],
                )

        def init_batch_tiles(b):
            st = state[b]
            st["q"] = pqk.tile([128, S], BF16, tag="qT", name=f"qT_{b}")
            st["k"] = pqk.tile([128, S], BF16, tag="kT", name=f"kT_{b}")
            st["v"] = pv.tile([128, S], BF16, tag="vT", name=f"vT_{b}")
            # vS: [keys, key-chunk, 130]; cols 0:64 = V head0, col 64 = ones,
            # cols 65:129 = V head1, col 129 = ones.
            st["vS"] = pv.tile([128, NKC, 130], BF16, tag="vS", name=f"vS_{b}")
            nc.vector.memset(st["vS"][:, :, 64:65], 1.0)
            nc.vector.memset(st["vS"][:, :, 129:130], 1.0)

        def qkv_fillers(b, name):
            """Per (tensor, q-chunk): 8 accumulating matmuls + DVE evac,
            one matmul per filler closure to smooth PE load."""
            st = state[b]
            w_t = w_sb[name]
            fills = []
            for qc in range(4):
                ps_ref = {}

                def mk(qc, ps_ref, kc):
                    def go():
                        if kc == 0:
                            ps_ref["ps"] = psQ.tile(
                                [128, 512], F32, tag="psQ", name="qkv_ps"
                            )
                        ps = ps_ref["ps"]
                        nc.tensor.matmul(
                            ps,
                            w_t[:, kc, :],
                            st["x"][kc][:, qc * 512 : (qc + 1) * 512],
                            start=(kc == 0),
                            stop=(not with_bias and kc == KC - 1),
                            skip_group_check=True,
                        )
                        if kc == KC - 1:
                            if with_bias:
                                nc.tensor.matmul(
                                    ps, b_sb[name], ones_row,
                                    start=False, stop=True,
                                    skip_group_check=True,
                                )
                            nc.vector.tensor_copy(
                                st[name][:, qc * 512 : (qc + 1) * 512], ps
                            )
                        return 216
                    return go

                for kc in range(KC):
                    fills.append(mk(qc, ps_ref, kc))
            return fills

        def vtrans_fillers(b):
            """16 transposes of vT into token-major vS (+1 DVE copy each)."""
            st = state[b]
            fills = []

            def mk(t):
                def go():
                    tp = psA.tile([128, 512], BF16, tag="psA", name="vtp")
                    nc.tensor.transpose(
                        tp[:, 0:128], st["v"][:, t * 128 : (t + 1) * 128], ident
                    )
                    # one copy: [128, 2, 64] strided dst (skip ones columns)
                    dst = st["vS"][:, t, :].rearrange(
                        "p (g i) -> p g i", g=2, i=65
                    )[:, :, 0:64]
                    src = tp[:, 0:128].rearrange("p (g i) -> p g i", g=2, i=64)
                    nc.vector.tensor_copy(dst, src)
                    return 140
                return go

            for t in range(NKC):
                fills.append(mk(t))
            return fills

        def outproj_fillers(b, qt):
            """Output projection for q-tile qt of batch b: per token tile,
            2 matmuls + 2 DVE evacs + 1 DMA, split into 2 closures."""
            st = state[b]
            aTq = st[f"aT{qt}"]
            fills = []
            if b == B - 1 and qt == 3:
                # last q-tile: M=1024 matmuls through the (by now idle)
                # score-psum pool for a short serial tail
                def mk_tail(lt):
                    def go():
                        po = psS.tile([128, 1024], F32, tag="psS")
                        for g in range(2):
                            nc.tensor.matmul(
                                po[:, g * 512 : (g + 1) * 512],
                                aTq[:, lt * 128 : (lt + 1) * 128],
                                wo_sb[:, g * 512 : (g + 1) * 512],
                                start=True,
                                stop=True,
                                skip_group_check=True,
                            )
                        ob = pob.tile([128, 1024], BF16, tag="ob", name="obt")
                        nc.vector.tensor_copy(ob, po)
                        t0 = b * S + qt * 512 + lt * 128
                        nc.gpsimd.dma_start(out[t0 : t0 + 128, :], ob)
                        return 432
                    return go

                for lt in range(4):
                    fills.append(mk_tail(lt))
                return fills
            for lt in range(4):
                ps_ref = {}

                def mk(lt, ps_ref, g):
                    def go():
                        if g == 0:
                            ps_ref["ob"] = pob.tile(
                                [128, 1024], BF16, tag="ob", name="ob"
                            )
                        po = psA.tile([128, 512], F32, tag="psA")
                        nc.tensor.matmul(
                            po,
                            aTq[:, lt * 128 : (lt + 1) * 128],
                            wo_sb[:, g * 512 : (g + 1) * 512],
                            start=True,
                            stop=True,
                            skip_group_check=True,
                        )
                        ob = ps_ref["ob"]
                        nc.vector.tensor_copy(
                            ob[:, g * 512 : (g + 1) * 512], po
                        )
                        if g == 1:
                            t0 = b * S + qt * 512 + lt * 128
                            nc.gpsimd.dma_start(out[t0 : t0 + 128, :], ob)
                        return 216
                    return go

                for g in range(2):
                    fills.append(mk(lt, ps_ref, g))
            return fills

        # ------------------------------------------------------------------
        # pipelined streams
        # ------------------------------------------------------------------
        def decode(s):
            return s // 64, (s % 64) // 16, s % 16  # b, qt, kc

        n_steps = B * 64

        def emit_scores(s):
            b, qt, kc = decode(s)
            st = state[b]
            qT, kT = st["q"], st["k"]
            q0, q1 = qt * 512, (qt + 1) * 512
            k0 = kc * 128
            sp = psS.tile([128, 1024], F32, tag="psS")
            nc.tensor.matmul(
                sp[:, 0:512], kT[0:64, k0 : k0 + 128], qT[0:64, q0:q1],
                start=True, stop=True, tile_position=(0, 0),
                skip_group_check=True,
            )
            nc.tensor.matmul(
                sp[:, 512:1024], kT[64:128, k0 : k0 + 128], qT[64:128, q0:q1],
                start=True, stop=True, tile_position=(64, 0),
                skip_group_check=True,
            )
            st.setdefault("sp", {})[s] = sp

        def emit_act(s):
            b, qt, kc = decode(s)
            st = state[b]
            sp = st["sp"].pop(s)
            pt = ppt.tile([128, 1024], BF16, tag="pt", name=f"pt_{s % (LAG + 2)}")
            nc.scalar.activation(pt, sp, EXP, scale=float(SCALE))
            st.setdefault("pt", {})[s] = pt

        def emit_av(s):
            b, qt, kc = decode(s)
            st = state[b]
            pt = st["pt"].pop(s)
            nc.tensor.matmul(
                u01[0:65, 0:512], st["vS"][:, kc, 0:65], pt[:, 0:512],
                start=(kc == 0), stop=(kc == NKC - 1),
                skip_group_check=True,
            )
            nc.tensor.matmul(
                u01[0:65, 512:1024], st["vS"][:, kc, 65:130], pt[:, 512:1024],
                start=(kc == 0), stop=(kc == NKC - 1),
                skip_group_check=True,
            )
            if kc == NKC - 1:
                emit_dchain(b, qt)

        def emit_dchain(b, qt):
            """Normalize q-tile qt of batch b out of u01 into aT{qt}.

            Emits only the two DVE reads of u01 inline (so u01 is free for
            the next q-tile after ~1.5us); the rest of the chain (transpose
            d, reciprocal, broadcast roundtrip, normalize-multiply) is
            deferred into a filler closure so the in-order PE queue never
            blocks on it."""
            st = state[b]
            # unnormalized U + denominator row out of PSUM in ONE DVE copy
            # (frees u01 for the next q-tile as fast as possible)
            usb = pd.tile([65, 1024], BF16, tag="usb", name="usb")
            nc.scalar.copy(usb, u01[0:65, :])
            dsb = usb[64:65, :]

            def chain():
                # transpose d into partition-parallel layout with 8 tiny
                # stationary-[1,128] matmuls against a [1,1] ones moving tile
                tcol = psA.tile([128, 512], F32, tag="psA", name="tcol")
                for j in range(8):
                    nc.tensor.matmul(
                        tcol[:, j : j + 1],
                        dsb[0:1, j * 128 : (j + 1) * 128],
                        ones1[64:65, :],
                        start=True,
                        stop=True,
                        skip_group_check=True,
                    )
                dinvT = pd.tile([128, 8], F32, tag="dinvT", name="dinvT")
                nc.vector.reciprocal(dinvT, tcol[:, 0:8])
                # transpose back via PE: [128, 8] -> [8, 128]
                t8 = psA.tile([128, 512], F32, tag="psA", name="t8")
                nc.tensor.transpose(t8[0:8, 0:128], dinvT, identf)
                # roundtrip through DRAM to broadcast across partitions
                t8sb = pd.tile([8, 128], BF16, tag="t8sb", name="t8sb")
                nc.vector.tensor_copy(t8sb, t8[0:8, 0:128])
                dscr = dsc.tile([1, 1024], BF16, tag="dscr", name="dscr")
                nc.sync.dma_start(
                    dscr[0:1, :].rearrange("a (j i) -> (a j) i", j=8, i=128),
                    t8sb,
                )
                bc = pbc.tile([64, 1024], BF16, tag="bc", name="bc")
                nc.sync.dma_start(bc, dscr.to_broadcast((64, 1024)))
                # normalize: aT[h*64:(h+1)*64, q] = U * (1/d)
                aTq = pa.tile([128, 512], BF16, tag="aT", name=f"aT_{b}_{qt}")
                st[f"aT{qt}"] = aTq
                nc.vector.tensor_mul(
                    aTq[0:64, :], usb[0:64, 0:512], bc[:, 0:512]
                )
                nc.vector.tensor_mul(
                    aTq[64:128, :], usb[0:64, 512:1024], bc[:, 512:1024]
                )
                for f in outproj_fillers(b, qt):
                    defer_q.append(f)
                return 700

            fill_q.append(chain)

        # ------------------------------------------------------------------
        # prologue: batch 0 — x DMAs, k(qc0) and q(qt0) inline; remaining
        # k-groups are emitted inline just before the scores that need
        # them (emission order is what creates sync deps, so a consumer
        # must never be emitted before its producer).
        # ------------------------------------------------------------------
        for _part in range(4):
            push_x_dmas(0, _part)
        init_batch_tiles(0)
        b0_k = qkv_fillers(0, "k")
        b0_q = qkv_fillers(0, "q")
        for f in b0_k[:8]:
            f()
        for f in b0_q[:8]:
            f()
        vf = qkv_fillers(0, "v")
        vt = vtrans_fillers(0)
        for qc in range(4):
            for f in vf[qc * 8 : (qc + 1) * 8]:
                fill_q.append(f)
            for f in vt[qc * 4 : (qc + 1) * 4]:
                fill_q.append(f)
            for f in b0_q[8 * (qc + 1) : 8 * (qc + 2)]:
                fill_q.append(f)

        # ------------------------------------------------------------------
        # main loop: step-pairs (2 score steps, 2 lagged AV steps, fillers).
        # The AV stream is a gated cursor: the kc==0 AV of each q-tile may
        # only be emitted one pair after the previous q-tile's d-chain, so
        # the in-order PE queue never blocks on the u01 evacuation.
        # ------------------------------------------------------------------
        av_state = {"next": 0, "gate": -1, "pair": 0}

        def pump_avs(limit):
            n = 0
            while av_state["next"] <= limit and n < 3:
                a = av_state["next"]
                kk = a % 16
                if kk == 0 and a > 0 and av_state["pair"] <= av_state["gate"]:
                    break
                emit_av(a)
                if kk == NKC - 1:
                    av_state["gate"] = av_state["pair"]
                av_state["next"] += 1
                n += 1
            return 2 * n

        for b in range(B):
            if b + 1 < B:
                push_x_dmas(b + 1, 0)
                push_x_dmas(b + 1, 1)
                init_batch_tiles(b + 1)
                for f in qkv_fillers(b + 1, "k"):
                    fill_q.append(f)
                vf = qkv_fillers(b + 1, "v")
                vt = vtrans_fillers(b + 1)
                for qc in range(4):
                    for f in vf[qc * 8 : (qc + 1) * 8]:
                        fill_q.append(f)
                    for f in vt[qc * 4 : (qc + 1) * 4]:
                        fill_q.append(f)
                for f in qkv_fillers(b + 1, "q"):
                    fill_q.append(f)
            for local in range(0, 64, 2):
                s = b * 64 + local
                if b + 1 < B and local == 4:
                    push_x_dmas(b + 1, 2)
                    push_x_dmas(b + 1, 3)
                if b == 0 and local in (4, 8, 12):
                    for f in b0_k[8 * (local // 4) : 8 * (local // 4 + 1)]:
                        f()
                emit_scores(s)
                emit_act(s)
                emit_scores(s + 1)
                emit_act(s + 1)
                lag = 2 if (b == B - 1 and local >= 48) else LAG
                n_av = pump_avs(s + 1 - lag)
                for _ in range(2 if b == B - 1 else 1):
                    if defer_q:
                        fill_q.append(defer_q.popleft())
                # act cadence per pair is ~2294ns; keep emitted PE work just
                # below it (scores pair ~432ns, each AV ~216ns)
                budget = 2250 - 432 - 216 * n_av
                if b == 0 and local < 24:
                    budget += 700  # prologue catch-up
                pop_fillers_budget(budget)
                av_state["pair"] += 1

        # tail: drain remaining AV steps and fillers
        while av_state["next"] < n_steps:
            pump_avs(n_steps - 1)
            pop_fillers(4)
            av_state["pair"] += 1
        drain_fillers()
        while defer_q:
            fill_q.append(defer_q.popleft())
        drain_fillers()

    return nc


_NC_CACHE = {}


def _get_nc(with_bias=False):
    key = with_bias
    if key not in _NC_CACHE:
        _NC_CACHE[key] = build_attention_nc(with_bias)
    return _NC_CACHE[key]


def _run(inputs, Wq, bq, Wk, bk, Wv, bv, Wo, bo, trace=False, **spmd_kwargs):
    X2 = np.asarray(inputs, dtype=np.float32).reshape(T, D)
    xT = X2.T.astype(NP_BF16).reshape(KC, 128, T)
    with_bias = bool(
        np.any(np.asarray(bq)) or np.any(np.asarray(bk)) or np.any(np.asarray(bv))
    )

    in_maps = []
    for c in range(N_CORES):
        cs = slice(c * 128, (c + 1) * 128)
        in_maps.append(
            {
                "xT": xT,
                "wq": np.ascontiguousarray(Wq[:, cs]).astype(NP_BF16).reshape(KC, 128, 128),
                "wk": np.ascontiguousarray(Wk[:, cs]).astype(NP_BF16).reshape(KC, 128, 128),
                "wv": np.ascontiguousarray(Wv[:, cs]).astype(NP_BF16).reshape(KC, 128, 128),
                "bq": np.asarray(bq[cs]).astype(NP_BF16),
                "bk": np.asarray(bk[cs]).astype(NP_BF16),
                "bv": np.asarray(bv[cs]).astype(NP_BF16),
                "wo": np.ascontiguousarray(Wo[cs, :]).astype(NP_BF16),
            }
        )

    if not with_bias:
        for m in in_maps:
            m.pop("bq"), m.pop("bk"), m.pop("bv")
    res = run_bass_kernel_spmd(
        _get_nc(with_bias), in_maps, list(range(N_CORES)), trace=trace, **spmd_kwargs
    )
    acc = res.results[0]["out"].astype(np.float32)
    for c in range(1, N_CORES):
        acc += res.results[c]["out"].astype(np.float32)
    acc += np.asarray(bo, dtype=np.float32)[None, :]
    return acc.reshape(B, S, D), res


def kernel(inputs, Wq, bq, Wk, bk, Wv, bv, Wo, bo):
    out, _ = _run(inputs, Wq, bq, Wk, bk, Wv, bv, Wo, bo)
    return out


# revision 33
# speedup vs baseline: 1.0638x; 1.0638x over previous
"""Multi-head self-attention (B=4, S=2048, D=1024, H=16) on 8 NeuronCores.

Sharding: tensor-parallel over heads. Core c owns heads {2c, 2c+1} = 128
columns of Wq/Wk/Wv and 128 rows of Wo. Each core computes Q^T/K^T/V for its
two heads over all tokens, runs attention for its 8 (batch, head) pairs, and
produces a partial output O_c = A_c @ Wo_c. The all-reduce over the 8
partials is done on the host during unsharding.

v2: software-pipelined at key-chunk granularity so the tensor engine never
stalls on the softmax exp (which would drop it from its 2.4 GHz ramped
p-state back to 1.2 GHz). The scalar engine runs only the exp activations
(the true bottleneck at ~285 us); QKV projections for the next batch,
V transposes, and the output projection of the previous q-tile are emitted
as tensor-engine fillers between score/AV matmuls. Softmax normalization is
restructured: denominators (from a ones-column in the V stationary) are
transposed into partition-parallel layout with tiny stationary-[1,128]
matmuls, reciprocated as [128,8] on the DVE (203ns vs 3.3us for [1,512]),
transposed back via the PE, and applied in a fused normalize+evacuate
tensor_tensor multiply.
"""
import os
import sys

for _p in ("/opt/trn_rl_repo", "/root/.axon_site/_ro/trn_rl_repo"):
    if os.path.isdir(_p) and _p not in sys.path:
        sys.path.append(_p)

from collections import deque
from contextlib import ExitStack

import numpy as np
import ml_dtypes

import concourse.bass as bass
import concourse.tile as tile
from concourse import mybir
from concourse.bass_utils import run_bass_kernel_spmd
from concourse.masks import make_identity

BF16 = mybir.dt.bfloat16
F32 = mybir.dt.float32
EXP = mybir.ActivationFunctionType.Exp
NP_BF16 = ml_dtypes.bfloat16

B, S, D = 4, 2048, 1024
H, HD = 16, 64
N_CORES = 8
T = B * S  # 8192 tokens
KC = D // 128  # 8 contraction chunks
NKC = S // 128  # 16 key chunks per batch
SCALE = 1.0 / np.sqrt(HD)
LAG = 5  # AV stream lags the score stream by this many kc-steps

# ---------------------------------------------------------------------------
# Tile patches: this walrus build rejects instructions with more than one
# sync wait ("Too many sync wait commands"), so split extra waits into
# preceding same-engine nops, and replace the kernel-tail drain's wait list
# with a chain of single-wait SP nops.
# ---------------------------------------------------------------------------
_MAX_WAITS = 1
_patched = False


def _install_tile_patches():
    global _patched
    if _patched:
        return
    _patched = True
    from concourse.vector_clock import ScopedClock, VectorClock

    orig_lower = tile.TileContext._lower_ordered_insts

    def split_inst_waits(self, ordered):
        for bb_name in list(ordered.keys()):
            insts = ordered[bb_name]
            new = []
            for inst in insts:
                si = inst.sync_info
                if si is not None and len(si.on_wait) > _MAX_WAITS:
                    waits = list(si.on_wait)
                    head, tail = waits[:-_MAX_WAITS], waits[-_MAX_WAITS:]
                    for w in head:
                        nop = mybir.InstNoOp(
                            name=f"ws-{self.nc.next_id()}",
                            engine=inst.engine,
                            bass_nofuse=True,
                        )
                        nop.sync_info = mybir.SyncInfo(on_wait=[w], on_update=[])
                        new.append(nop)
                    inst.sync_info = mybir.SyncInfo(
                        on_wait=tail, on_update=list(si.on_update)
                    )
                new.append(inst)
            ordered[bb_name] = new
        return orig_lower(self, ordered)

    def split_drain_and_barrier(self, tick_clock, wait_clock):
        gc = tick_clock.global_clock
        ticks = eval(repr(gc).replace("VectorClock", ""))
        procs = [(i, t) for i, t in enumerate(ticks) if t > 0]
        for i in range(0, len(procs), _MAX_WAITS):
            chunk = procs[i : i + _MAX_WAITS]
            nop = self.nc.sync.nop(nofuse=True, hint="drain_wait_split")
            pc = VectorClock()
            for proc, tick in chunk:
                pc.require_at_least(proc, tick)
            wait_clock.add_sem_waits(nop.ins, ScopedClock({None: pc}))
        drain_inst = self.nc.sync.drain()
        wait_clock.add_sem_waits(
            drain_inst.ins, ScopedClock({None: gc}), ScopedClock({None: gc.copy()})
        )
        self.nc.all_engine_barrier()
        assert self.sems is not None
        popped = self.nc._tile_sem_poison_stack.pop()
        assert popped is self._sem_poison
        self.nc.clear_and_free_semaphores(list(self.sems.allocated().values()))
        self.nc.all_engine_barrier()

    tile.TileContext._lower_ordered_insts = split_inst_waits
    tile.TileContext._drain_and_barrier = split_drain_and_barrier


# ---------------------------------------------------------------------------
# Device kernel
# ---------------------------------------------------------------------------
def build_attention_nc(with_bias=False):
    _install_tile_patches()
    nc = bass.Bass()

    xT = nc.declare_dram_parameter("xT", [KC, 128, T], BF16, isOutput=False)
    wq = nc.declare_dram_parameter("wq", [KC, 128, 128], BF16, isOutput=False)
    wk = nc.declare_dram_parameter("wk", [KC, 128, 128], BF16, isOutput=False)
    wv = nc.declare_dram_parameter("wv", [KC, 128, 128], BF16, isOutput=False)
    if with_bias:
        bq = nc.declare_dram_parameter("bq", [128], BF16, isOutput=False)
        bk = nc.declare_dram_parameter("bk", [128], BF16, isOutput=False)
        bv = nc.declare_dram_parameter("bv", [128], BF16, isOutput=False)
    else:
        bq = bk = bv = None
    wo = nc.declare_dram_parameter("wo", [128, D], BF16, isOutput=False)
    out = nc.declare_dram_parameter("out", [T, D], BF16, isOutput=True)

    with tile.TileContext(nc) as tc, ExitStack() as ctx:
        singles = ctx.enter_context(tc.tile_pool(name="singles", bufs=1))
        px = ctx.enter_context(tc.tile_pool(name="px", bufs=16))
        pqk = ctx.enter_context(tc.tile_pool(name="pqk", bufs=2))
        pv = ctx.enter_context(tc.tile_pool(name="pv", bufs=2))
        ppt = ctx.enter_context(tc.tile_pool(name="ppt", bufs=10))
        pa = ctx.enter_context(tc.tile_pool(name="pa", bufs=4))
        pob = ctx.enter_context(tc.tile_pool(name="pob", bufs=3))
        pd = ctx.enter_context(tc.tile_pool(name="pd", bufs=2))
        pbc = ctx.enter_context(tc.tile_pool(name="pbc", bufs=3))
        dsc = ctx.enter_context(tc.tile_pool(name="dsc", bufs=4, space="DRAM"))
        psS = ctx.enter_context(tc.tile_pool(name="psS", bufs=2, space="PSUM"))
        psU = ctx.enter_context(tc.tile_pool(name="psU", bufs=1, space="PSUM"))
        psQ = ctx.enter_context(tc.tile_pool(name="psQ", bufs=1, space="PSUM"))
        psA = ctx.enter_context(tc.tile_pool(name="psA", bufs=1, space="PSUM"))

        # --- constants / weights, loaded once -----------------------------
        w_sb = {}
        b_sb = {}
        for name, wd, bd in (("q", wq, bq), ("k", wk, bk), ("v", wv, bv)):
            w_t = singles.tile([128, KC, 128], BF16, tag=f"w{name}")
            nc.sync.dma_start(w_t, wd[:, :, :].rearrange("k p m -> p k m"))
            w_sb[name] = w_t
            if with_bias:
                b_t = singles.tile([1, 128], BF16, tag=f"b{name}")
                nc.sync.dma_start(b_t, bd[:][None, :])
                b_sb[name] = b_t
        wo_sb = singles.tile([128, D], BF16, tag="wo")
        nc.sync.dma_start(wo_sb, wo[:, :])
        ones_row = singles.tile([1, 512], BF16, tag="ones_row")
        nc.vector.memset(ones_row, 1.0)
        ones1 = singles.tile([128, 1], BF16, tag="ones1")
        nc.vector.memset(ones1, 1.0)
        ident = singles.tile([128, 128], BF16, tag="ident")
        make_identity(nc, ident)
        identf = singles.tile([128, 128], F32, tag="identf")
        make_identity(nc, identf)

        # u01: single PSUM accumulator [128, 1024] f32; head0 in cols 0:512,
        # head1 in cols 512:1024; row 64 collects softmax denominators via
        # the ones column (index 64 resp. 129) in the vS stationary.
        u01 = psU.tile([128, 1024], F32, tag="u01")

        state = [dict() for _ in range(B)]

        # ------------------------------------------------------------------
        # filler machinery: small closures, each ~1-2 PE instructions (plus
        # attached DVE/DMA work). Popped between score/AV steps.
        # ------------------------------------------------------------------
        fill_q = deque()
        defer_q = deque()

        def pop_fillers_budget(budget_ns):
            # fillers return their PE cost (ns); pop until budget exhausted
            spent = 0
            while fill_q and spent < budget_ns:
                spent += fill_q.popleft()() or 216
            return spent

        def pop_fillers(n):
            for _ in range(n):
                if not fill_q:
                    return
                fill_q.popleft()()

        def drain_fillers():
            while fill_q:
                fill_q.popleft()()

        # --- QKV projection / V transpose fillers for batch b -------------
        def push_x_dmas(b, part=0):
            """part 0: allocate tiles + first token-quarter slices (needed
            first by the QKV fillers); parts 1..3: remaining quarters,
            staggered so the Sync queue is never occupied for long."""
            st = state[b]
            if part == 0:
                st["x"] = [
                    px.tile([128, S], BF16, tag="x", name=f"x_{b}_{kc}")
                    for kc in range(KC)
                ]
            lo, hi = part * 512, (part + 1) * 512
            eng = nc.sync if part < 2 else nc.gpsimd
            for kc in range(KC):
                eng.dma_start(
                    st["x"][kc][:, lo:hi],
                    xT[kc, :, b * S + lo : b * S + hi],
                )

        def init_batch_tiles(b):
            st = state[b]
            st["q"] = pqk.tile([128, S], BF16, tag="qT", name=f"qT_{b}")
            st["k"] = pqk.tile([128, S], BF16, tag="kT", name=f"kT_{b}")
            st["v"] = pv.tile([128, S], BF16, tag="vT", name=f"vT_{b}")
            # vS: [keys, key-chunk, 130]; cols 0:64 = V head0, col 64 = ones,
            # cols 65:129 = V head1, col 129 = ones.
            st["vS"] = pv.tile([128, NKC, 130], BF16, tag="vS", name=f"vS_{b}")
            nc.vector.memset(st["vS"][:, :, 64:65], 1.0)
            nc.vector.memset(st["vS"][:, :, 129:130], 1.0)

        def qkv_fillers(b, name):
            """Per (tensor, q-chunk): 8 accumulating matmuls + DVE evac,
            one matmul per filler closure to smooth PE load."""
            st = state[b]
            w_t = w_sb[name]
            fills = []
            for qc in range(4):
                ps_ref = {}

                def mk(qc, ps_ref, kc):
                    def go():
                        if kc == 0:
                            ps_ref["ps"] = psQ.tile(
                                [128, 512], F32, tag="psQ", name="qkv_ps"
                            )
                        ps = ps_ref["ps"]
                        nc.tensor.matmul(
                            ps,
                            w_t[:, kc, :],
                            st["x"][kc][:, qc * 512 : (qc + 1) * 512],
                            start=(kc == 0),
                            stop=(not with_bias and kc == KC - 1),
                            skip_group_check=True,
                        )
                        if kc == KC - 1:
                            if with_bias:
                                nc.tensor.matmul(
                                    ps, b_sb[name], ones_row,
                                    start=False, stop=True,
                                    skip_group_check=True,
                                )
                            nc.vector.tensor_copy(
                                st[name][:, qc * 512 : (qc + 1) * 512], ps
                            )
                        return 216
                    return go

                for kc in range(KC):
                    fills.append(mk(qc, ps_ref, kc))
            return fills

        def vtrans_fillers(b):
            """16 transposes of vT into token-major vS (+1 DVE copy each)."""
            st = state[b]
            fills = []

            def mk(t):
                def go():
                    tp = psA.tile([128, 512], BF16, tag="psA", name="vtp")
                    nc.tensor.transpose(
                        tp[:, 0:128], st["v"][:, t * 128 : (t + 1) * 128], ident
                    )
                    # one copy: [128, 2, 64] strided dst (skip ones columns)
                    dst = st["vS"][:, t, :].rearrange(
                        "p (g i) -> p g i", g=2, i=65
                    )[:, :, 0:64]
                    src = tp[:, 0:128].rearrange("p (g i) -> p g i", g=2, i=64)
                    nc.vector.tensor_copy(dst, src)
                    return 140
                return go

            for t in range(NKC):
                fills.append(mk(t))
            return fills

        def outproj_fillers(b, qt):
            """Output projection for q-tile qt of batch b: per token tile,
            2 matmuls + 2 DVE evacs + 1 DMA, split into 2 closures."""
            st = state[b]
            aTq = st[f"aT{qt}"]
            fills = []
            if b == B - 1 and qt == 3:
                # last q-tile: M=1024 matmuls through the (by now idle)
                # score-psum pool for a short serial tail
                def mk_tail(lt):
                    def go():
                        po = psS.tile([128, 1024], F32, tag="psS")
                        for g in range(2):
                            nc.tensor.matmul(
                                po[:, g * 512 : (g + 1) * 512],
                                aTq[:, lt * 128 : (lt + 1) * 128],
                                wo_sb[:, g * 512 : (g + 1) * 512],
                                start=True,
                                stop=True,
                                skip_group_check=True,
                            )
                        ob = pob.tile([128, 1024], BF16, tag="ob", name="obt")
                        nc.vector.tensor_copy(ob, po)
                        t0 = b * S + qt * 512 + lt * 128
                        nc.gpsimd.dma_start(out[t0 : t0 + 128, :], ob)
                        return 432
                    return go

                for lt in range(4):
                    fills.append(mk_tail(lt))
                return fills
            for lt in range(4):
                ps_ref = {}

                def mk(lt, ps_ref, g):
                    def go():
                        if g == 0:
                            ps_ref["ob"] = pob.tile(
                                [128, 1024], BF16, tag="ob", name="ob"
                            )
                        po = psA.tile([128, 512], F32, tag="psA")
                        nc.tensor.matmul(
                            po,
                            aTq[:, lt * 128 : (lt + 1) * 128],
                            wo_sb[:, g * 512 : (g + 1) * 512],
                            start=True,
                            stop=True,
                            skip_group_check=True,
                        )
                        ob = ps_ref["ob"]
                        nc.vector.tensor_copy(
                            ob[:, g * 512 : (g + 1) * 512], po
                        )
                        if g == 1:
                            t0 = b * S + qt * 512 + lt * 128
                            nc.gpsimd.dma_start(out[t0 : t0 + 128, :], ob)
                        return 216
                    return go

                for g in range(2):
                    fills.append(mk(lt, ps_ref, g))
            return fills

        # ------------------------------------------------------------------
        # pipelined streams
        # ------------------------------------------------------------------
        def decode(s):
            return s // 64, (s % 64) // 16, s % 16  # b, qt, kc

        n_steps = B * 64

        def emit_scores(s):
            b, qt, kc = decode(s)
            st = state[b]
            qT, kT = st["q"], st["k"]
            q0, q1 = qt * 512, (qt + 1) * 512
            k0 = kc * 128
            sp = psS.tile([128, 1024], F32, tag="psS")
            nc.tensor.matmul(
                sp[:, 0:512], kT[0:64, k0 : k0 + 128], qT[0:64, q0:q1],
                start=True, stop=True, tile_position=(0, 0),
                skip_group_check=True,
            )
            nc.tensor.matmul(
                sp[:, 512:1024], kT[64:128, k0 : k0 + 128], qT[64:128, q0:q1],
                start=True, stop=True, tile_position=(64, 0),
                skip_group_check=True,
            )
            st.setdefault("sp", {})[s] = sp

        def emit_act(s):
            b, qt, kc = decode(s)
            st = state[b]
            sp = st["sp"].pop(s)
            pt = ppt.tile([128, 1024], BF16, tag="pt", name=f"pt_{s % 10}")
            nc.scalar.activation(pt, sp, EXP, scale=float(SCALE))
            st.setdefault("pt", {})[s] = pt

        def emit_av(s):
            b, qt, kc = decode(s)
            st = state[b]
            pt = st["pt"].pop(s)
            nc.tensor.matmul(
                u01[0:65, 0:512], st["vS"][:, kc, 0:65], pt[:, 0:512],
                start=(kc == 0), stop=(kc == NKC - 1),
                skip_group_check=True,
            )
            nc.tensor.matmul(
                u01[0:65, 512:1024], st["vS"][:, kc, 65:130], pt[:, 512:1024],
                start=(kc == 0), stop=(kc == NKC - 1),
                skip_group_check=True,
            )
            if kc == NKC - 1:
                emit_dchain(b, qt)

        def emit_dchain(b, qt):
            """Normalize q-tile qt of batch b out of u01 into aT{qt}.

            Emits only the two DVE reads of u01 inline (so u01 is free for
            the next q-tile after ~1.5us); the rest of the chain (transpose
            d, reciprocal, broadcast roundtrip, normalize-multiply) is
            deferred into a filler closure so the in-order PE queue never
            blocks on it."""
            st = state[b]
            # unnormalized U + denominator row out of PSUM in ONE DVE copy
            # (frees u01 for the next q-tile as fast as possible)
            usb = pd.tile([65, 1024], BF16, tag="usb", name="usb", bufs=3)
            nc.scalar.copy(usb, u01[0:65, :])
            dsb = usb[64:65, :]

            def chain():
                # transpose d into partition-parallel layout with 8 tiny
                # stationary-[1,128] matmuls against a [1,1] ones moving tile
                tcol = psA.tile([128, 512], F32, tag="psA", name="tcol")
                for j in range(8):
                    nc.tensor.matmul(
                        tcol[:, j : j + 1],
                        dsb[0:1, j * 128 : (j + 1) * 128],
                        ones1[64:65, :],
                        start=True,
                        stop=True,
                        skip_group_check=True,
                    )
                dinvT = pd.tile([128, 8], F32, tag="dinvT", name="dinvT")
                nc.vector.reciprocal(dinvT, tcol[:, 0:8])
                # transpose back via PE: [128, 8] -> [8, 128]
                t8 = psA.tile([128, 512], F32, tag="psA", name="t8")
                nc.tensor.transpose(t8[0:8, 0:128], dinvT, identf)
                # roundtrip through DRAM to broadcast across partitions
                t8sb = pd.tile([8, 128], BF16, tag="t8sb", name="t8sb")
                nc.vector.tensor_copy(t8sb, t8[0:8, 0:128])
                dscr = dsc.tile([1, 1024], BF16, tag="dscr", name="dscr")
                nc.sync.dma_start(
                    dscr[0:1, :].rearrange("a (j i) -> (a j) i", j=8, i=128),
                    t8sb,
                )
                bc = pbc.tile([64, 1024], BF16, tag="bc", name="bc")
                nc.sync.dma_start(bc, dscr.to_broadcast((64, 1024)))
                # normalize: aT[h*64:(h+1)*64, q] = U * (1/d)
                aTq = pa.tile([128, 512], BF16, tag="aT", name=f"aT_{b}_{qt}")
                st[f"aT{qt}"] = aTq
                nc.vector.tensor_mul(
                    aTq[0:64, :], usb[0:64, 0:512], bc[:, 0:512]
                )
                nc.vector.tensor_mul(
                    aTq[64:128, :], usb[0:64, 512:1024], bc[:, 512:1024]
                )
                for f in outproj_fillers(b, qt):
                    defer_q.append(f)
                return 700

            fill_q.appendleft(chain)

        # ------------------------------------------------------------------
        # prologue: batch 0 — x DMAs, k(qc0) and q(qt0) inline; remaining
        # k-groups are emitted inline just before the scores that need
        # them (emission order is what creates sync deps, so a consumer
        # must never be emitted before its producer).
        # ------------------------------------------------------------------
        for _part in range(4):
            push_x_dmas(0, _part)
        init_batch_tiles(0)
        b0_k = qkv_fillers(0, "k")
        b0_q = qkv_fillers(0, "q")
        for f in b0_k[:8]:
            f()
        for f in b0_q[:8]:
            f()
        vf = qkv_fillers(0, "v")
        vt = vtrans_fillers(0)
        for qc in range(4):
            for f in vf[qc * 8 : (qc + 1) * 8]:
                fill_q.append(f)
            for f in vt[qc * 4 : (qc + 1) * 4]:
                fill_q.append(f)
            for f in b0_q[8 * (qc + 1) : 8 * (qc + 2)]:
                fill_q.append(f)

        # ------------------------------------------------------------------
        # main loop: step-pairs (2 score steps, 2 lagged AV steps, fillers).
        # The AV stream is a gated cursor: the kc==0 AV of each q-tile may
        # only be emitted one pair after the previous q-tile's d-chain, so
        # the in-order PE queue never blocks on the u01 evacuation.
        # ------------------------------------------------------------------
        av_state = {"next": 0, "gate": -1, "pair": 0}

        def pump_avs(limit):
            n = 0
            while av_state["next"] <= limit and n < 3:
                a = av_state["next"]
                kk = a % 16
                if kk == 0 and a > 0 and av_state["pair"] <= av_state["gate"]:
                    break
                emit_av(a)
                if kk == NKC - 1:
                    av_state["gate"] = av_state["pair"]
                av_state["next"] += 1
                n += 1
            return 2 * n

        for b in range(B):
            if b + 1 < B:
                push_x_dmas(b + 1, 0)
                push_x_dmas(b + 1, 1)
                init_batch_tiles(b + 1)
                for f in qkv_fillers(b + 1, "k"):
                    fill_q.append(f)
                vf = qkv_fillers(b + 1, "v")
                vt = vtrans_fillers(b + 1)
                for qc in range(4):
                    for f in vf[qc * 8 : (qc + 1) * 8]:
                        fill_q.append(f)
                    for f in vt[qc * 4 : (qc + 1) * 4]:
                        fill_q.append(f)
                for f in qkv_fillers(b + 1, "q"):
                    fill_q.append(f)
            for local in range(0, 64, 2):
                s = b * 64 + local
                if b + 1 < B and local == 4:
                    push_x_dmas(b + 1, 2)
                    push_x_dmas(b + 1, 3)
                if b == 0 and local in (4, 8, 12):
                    for f in b0_k[8 * (local // 4) : 8 * (local // 4 + 1)]:
                        f()
                emit_scores(s)
                emit_act(s)
                emit_scores(s + 1)
                emit_act(s + 1)
                lag = 2 if (b == B - 1 and local >= 48) else LAG
                n_av = pump_avs(s + 1 - lag)
                for _ in range(2 if b == B - 1 else 1):
                    if defer_q:
                        fill_q.append(defer_q.popleft())
                # act cadence per pair is ~2294ns; keep emitted PE work just
                # below it (scores pair ~432ns, each AV ~216ns)
                budget = 2250 - 432 - 216 * n_av
                if b == 0 and local < 24:
                    budget += 700  # prologue catch-up
                pop_fillers_budget(budget)
                av_state["pair"] += 1

        # tail: drain remaining AV steps and fillers
        while av_state["next"] < n_steps:
            pump_avs(n_steps - 1)
            pop_fillers(4)
            av_state["pair"] += 1
        drain_fillers()
        while defer_q:
            fill_q.append(defer_q.popleft())
        drain_fillers()

    return nc


_NC_CACHE = {}


def _get_nc(with_bias=False):
    key = with_bias
    if key not in _NC_CACHE:
        _NC_CACHE[key] = build_attention_nc(with_bias)
    return _NC_CACHE[key]


def _run(inputs, Wq, bq, Wk, bk, Wv, bv, Wo, bo, trace=False, **spmd_kwargs):
    X2 = np.asarray(inputs, dtype=np.float32).reshape(T, D)
    xT = X2.T.astype(NP_BF16).reshape(KC, 128, T)
    with_bias = bool(
        np.any(np.asarray(bq)) or np.any(np.asarray(bk)) or np.any(np.asarray(bv))
    )

    in_maps = []
    for c in range(N_CORES):
        cs = slice(c * 128, (c + 1) * 128)
        in_maps.append(
            {
                "xT": xT,
                "wq": np.ascontiguousarray(Wq[:, cs]).astype(NP_BF16).reshape(KC, 128, 128),
                "wk": np.ascontiguousarray(Wk[:, cs]).astype(NP_BF16).reshape(KC, 128, 128),
                "wv": np.ascontiguousarray(Wv[:, cs]).astype(NP_BF16).reshape(KC, 128, 128),
                "bq": np.asarray(bq[cs]).astype(NP_BF16),
                "bk": np.asarray(bk[cs]).astype(NP_BF16),
                "bv": np.asarray(bv[cs]).astype(NP_BF16),
                "wo": np.ascontiguousarray(Wo[cs, :]).astype(NP_BF16),
            }
        )

    if not with_bias:
        for m in in_maps:
            m.pop("bq"), m.pop("bk"), m.pop("bv")
    res = run_bass_kernel_spmd(
        _get_nc(with_bias), in_maps, list(range(N_CORES)), trace=trace, **spmd_kwargs
    )
    acc = res.results[0]["out"].astype(np.float32)
    for c in range(1, N_CORES):
        acc += res.results[c]["out"].astype(np.float32)
    acc += np.asarray(bo, dtype=np.float32)[None, :]
    return acc.reshape(B, S, D), res


def kernel(inputs, Wq, bq, Wk, bk, Wv, bv, Wo, bo):
    out, _ = _run(inputs, Wq, bq, Wk, bk, Wv, bv, Wo, bo)
    return out


# revision 35
# speedup vs baseline: 1.0771x; 1.0125x over previous
"""Multi-head self-attention (B=4, S=2048, D=1024, H=16) on 8 NeuronCores.

Sharding: tensor-parallel over heads. Core c owns heads {2c, 2c+1} = 128
columns of Wq/Wk/Wv and 128 rows of Wo. Each core computes Q^T/K^T/V for its
two heads over all tokens, runs attention for its 8 (batch, head) pairs, and
produces a partial output O_c = A_c @ Wo_c. The all-reduce over the 8
partials is done on the host during unsharding.

v2: software-pipelined at key-chunk granularity so the tensor engine never
stalls on the softmax exp (which would drop it from its 2.4 GHz ramped
p-state back to 1.2 GHz). The scalar engine runs only the exp activations
(the true bottleneck at ~285 us); QKV projections for the next batch,
V transposes, and the output projection of the previous q-tile are emitted
as tensor-engine fillers between score/AV matmuls. Softmax normalization is
restructured: denominators (from a ones-column in the V stationary) are
transposed into partition-parallel layout with tiny stationary-[1,128]
matmuls, reciprocated as [128,8] on the DVE (203ns vs 3.3us for [1,512]),
transposed back via the PE, and applied in a fused normalize+evacuate
tensor_tensor multiply.
"""
import os
import sys

for _p in ("/opt/trn_rl_repo", "/root/.axon_site/_ro/trn_rl_repo"):
    if os.path.isdir(_p) and _p not in sys.path:
        sys.path.append(_p)

from collections import deque
from contextlib import ExitStack

import numpy as np
import ml_dtypes

import concourse.bass as bass
import concourse.tile as tile
from concourse import mybir
from concourse.bass_utils import run_bass_kernel_spmd
from concourse.masks import make_identity

BF16 = mybir.dt.bfloat16
F32 = mybir.dt.float32
EXP = mybir.ActivationFunctionType.Exp
NP_BF16 = ml_dtypes.bfloat16

B, S, D = 4, 2048, 1024
H, HD = 16, 64
N_CORES = 8
T = B * S  # 8192 tokens
KC = D // 128  # 8 contraction chunks
NKC = S // 128  # 16 key chunks per batch
SCALE = 1.0 / np.sqrt(HD)
LAG = 5  # AV stream lags the score stream by this many kc-steps

# ---------------------------------------------------------------------------
# Tile patches: this walrus build rejects instructions with more than one
# sync wait ("Too many sync wait commands"), so split extra waits into
# preceding same-engine nops, and replace the kernel-tail drain's wait list
# with a chain of single-wait SP nops.
# ---------------------------------------------------------------------------
_MAX_WAITS = 1
_patched = False


def _install_tile_patches():
    global _patched
    if _patched:
        return
    _patched = True
    from concourse.vector_clock import ScopedClock, VectorClock

    orig_lower = tile.TileContext._lower_ordered_insts

    def split_inst_waits(self, ordered):
        for bb_name in list(ordered.keys()):
            insts = ordered[bb_name]
            new = []
            for inst in insts:
                si = inst.sync_info
                if si is not None and len(si.on_wait) > _MAX_WAITS:
                    waits = list(si.on_wait)
                    head, tail = waits[:-_MAX_WAITS], waits[-_MAX_WAITS:]
                    for w in head:
                        nop = mybir.InstNoOp(
                            name=f"ws-{self.nc.next_id()}",
                            engine=inst.engine,
                            bass_nofuse=True,
                        )
                        nop.sync_info = mybir.SyncInfo(on_wait=[w], on_update=[])
                        new.append(nop)
                    inst.sync_info = mybir.SyncInfo(
                        on_wait=tail, on_update=list(si.on_update)
                    )
                new.append(inst)
            ordered[bb_name] = new
        return orig_lower(self, ordered)

    def split_drain_and_barrier(self, tick_clock, wait_clock):
        gc = tick_clock.global_clock
        ticks = eval(repr(gc).replace("VectorClock", ""))
        procs = [(i, t) for i, t in enumerate(ticks) if t > 0]
        for i in range(0, len(procs), _MAX_WAITS):
            chunk = procs[i : i + _MAX_WAITS]
            nop = self.nc.sync.nop(nofuse=True, hint="drain_wait_split")
            pc = VectorClock()
            for proc, tick in chunk:
                pc.require_at_least(proc, tick)
            wait_clock.add_sem_waits(nop.ins, ScopedClock({None: pc}))
        drain_inst = self.nc.sync.drain()
        wait_clock.add_sem_waits(
            drain_inst.ins, ScopedClock({None: gc}), ScopedClock({None: gc.copy()})
        )
        self.nc.all_engine_barrier()
        assert self.sems is not None
        popped = self.nc._tile_sem_poison_stack.pop()
        assert popped is self._sem_poison
        self.nc.clear_and_free_semaphores(list(self.sems.allocated().values()))
        self.nc.all_engine_barrier()

    tile.TileContext._lower_ordered_insts = split_inst_waits
    tile.TileContext._drain_and_barrier = split_drain_and_barrier


# ---------------------------------------------------------------------------
# Device kernel
# ---------------------------------------------------------------------------
def build_attention_nc(with_bias=False):
    _install_tile_patches()
    nc = bass.Bass()

    xT = nc.declare_dram_parameter("xT", [KC, 128, T], BF16, isOutput=False)
    wq = nc.declare_dram_parameter("wq", [128, KC, 128], BF16, isOutput=False)
    wk = nc.declare_dram_parameter("wk", [128, KC, 128], BF16, isOutput=False)
    wv = nc.declare_dram_parameter("wv", [128, KC, 128], BF16, isOutput=False)
    if with_bias:
        bq = nc.declare_dram_parameter("bq", [128], BF16, isOutput=False)
        bk = nc.declare_dram_parameter("bk", [128], BF16, isOutput=False)
        bv = nc.declare_dram_parameter("bv", [128], BF16, isOutput=False)
    else:
        bq = bk = bv = None
    wo = nc.declare_dram_parameter("wo", [128, D], BF16, isOutput=False)
    out = nc.declare_dram_parameter("out", [T, D], BF16, isOutput=True)

    with tile.TileContext(nc) as tc, ExitStack() as ctx:
        singles = ctx.enter_context(tc.tile_pool(name="singles", bufs=1))
        px = ctx.enter_context(tc.tile_pool(name="px", bufs=16))
        pqk = ctx.enter_context(tc.tile_pool(name="pqk", bufs=2))
        pv = ctx.enter_context(tc.tile_pool(name="pv", bufs=2))
        ppt = ctx.enter_context(tc.tile_pool(name="ppt", bufs=10))
        pa = ctx.enter_context(tc.tile_pool(name="pa", bufs=4))
        pob = ctx.enter_context(tc.tile_pool(name="pob", bufs=3))
        pd = ctx.enter_context(tc.tile_pool(name="pd", bufs=2))
        pbc = ctx.enter_context(tc.tile_pool(name="pbc", bufs=3))
        dsc = ctx.enter_context(tc.tile_pool(name="dsc", bufs=4, space="DRAM"))
        psS = ctx.enter_context(tc.tile_pool(name="psS", bufs=2, space="PSUM"))
        psU = ctx.enter_context(tc.tile_pool(name="psU", bufs=1, space="PSUM"))
        psQ = ctx.enter_context(tc.tile_pool(name="psQ", bufs=1, space="PSUM"))
        psA = ctx.enter_context(tc.tile_pool(name="psA", bufs=1, space="PSUM"))

        # --- constants / weights, loaded once -----------------------------
        w_sb = {}
        b_sb = {}
        for name, wd, bd in (("k", wk, bk), ("q", wq, bq), ("v", wv, bv)):
            w_t = singles.tile([128, KC, 128], BF16, tag=f"w{name}")
            eng = nc.sync if name == "k" else nc.gpsimd
            eng.dma_start(w_t, wd[:, :, :])
            w_sb[name] = w_t
            if with_bias:
                b_t = singles.tile([1, 128], BF16, tag=f"b{name}")
                nc.sync.dma_start(b_t, bd[:][None, :])
                b_sb[name] = b_t
        wo_sb = singles.tile([128, D], BF16, tag="wo")
        nc.gpsimd.dma_start(wo_sb, wo[:, :])
        ones_row = singles.tile([1, 512], BF16, tag="ones_row")
        nc.vector.memset(ones_row, 1.0)
        ones1 = singles.tile([128, 1], BF16, tag="ones1")
        nc.vector.memset(ones1, 1.0)
        ident = singles.tile([128, 128], BF16, tag="ident")
        make_identity(nc, ident)
        identf = singles.tile([128, 128], F32, tag="identf")
        make_identity(nc, identf)

        # u01: single PSUM accumulator [128, 1024] f32; head0 in cols 0:512,
        # head1 in cols 512:1024; row 64 collects softmax denominators via
        # the ones column (index 64 resp. 129) in the vS stationary.
        u01 = psU.tile([128, 1024], F32, tag="u01")

        state = [dict() for _ in range(B)]

        # ------------------------------------------------------------------
        # filler machinery: small closures, each ~1-2 PE instructions (plus
        # attached DVE/DMA work). Popped between score/AV steps.
        # ------------------------------------------------------------------
        fill_q = deque()
        defer_q = deque()

        def pop_fillers_budget(budget_ns):
            # fillers return their PE cost (ns); pop until budget exhausted
            spent = 0
            while fill_q and spent < budget_ns:
                spent += fill_q.popleft()() or 216
            return spent

        def pop_fillers(n):
            for _ in range(n):
                if not fill_q:
                    return
                fill_q.popleft()()

        def drain_fillers():
            while fill_q:
                fill_q.popleft()()

        # --- QKV projection / V transpose fillers for batch b -------------
        def push_x_dmas(b, part=0):
            """part 0: allocate tiles + first token-quarter slices (needed
            first by the QKV fillers); parts 1..3: remaining quarters,
            staggered so the Sync queue is never occupied for long."""
            st = state[b]
            if part == 0:
                st["x"] = [
                    px.tile([128, S], BF16, tag="x", name=f"x_{b}_{kc}")
                    for kc in range(KC)
                ]
            lo, hi = part * 512, (part + 1) * 512
            eng = nc.sync if part < 2 else nc.gpsimd
            for kc in range(KC):
                eng.dma_start(
                    st["x"][kc][:, lo:hi],
                    xT[kc, :, b * S + lo : b * S + hi],
                )

        def init_batch_tiles(b):
            st = state[b]
            st["q"] = pqk.tile([128, S], BF16, tag="qT", name=f"qT_{b}")
            st["k"] = pqk.tile([128, S], BF16, tag="kT", name=f"kT_{b}")
            st["v"] = pv.tile([128, S], BF16, tag="vT", name=f"vT_{b}")
            # vS: [keys, key-chunk, 130]; cols 0:64 = V head0, col 64 = ones,
            # cols 65:129 = V head1, col 129 = ones.
            st["vS"] = pv.tile([128, NKC, 130], BF16, tag="vS", name=f"vS_{b}")
            nc.vector.memset(st["vS"][:, :, 64:65], 1.0)
            nc.vector.memset(st["vS"][:, :, 129:130], 1.0)

        def qkv_fillers(b, name):
            """Per (tensor, q-chunk): 8 accumulating matmuls + DVE evac,
            one matmul per filler closure to smooth PE load."""
            st = state[b]
            w_t = w_sb[name]
            fills = []
            for qc in range(4):
                ps_ref = {}

                def mk(qc, ps_ref, kc):
                    def go():
                        if kc == 0:
                            ps_ref["ps"] = psQ.tile(
                                [128, 512], F32, tag="psQ", name="qkv_ps"
                            )
                        ps = ps_ref["ps"]
                        nc.tensor.matmul(
                            ps,
                            w_t[:, kc, :],
                            st["x"][kc][:, qc * 512 : (qc + 1) * 512],
                            start=(kc == 0),
                            stop=(not with_bias and kc == KC - 1),
                            skip_group_check=True,
                        )
                        if kc == KC - 1:
                            if with_bias:
                                nc.tensor.matmul(
                                    ps, b_sb[name], ones_row,
                                    start=False, stop=True,
                                    skip_group_check=True,
                                )
                            nc.vector.tensor_copy(
                                st[name][:, qc * 512 : (qc + 1) * 512], ps
                            )
                        return 216
                    return go

                for kc in range(KC):
                    fills.append(mk(qc, ps_ref, kc))
            return fills

        def vtrans_fillers(b):
            """16 transposes of vT into token-major vS (+1 DVE copy each)."""
            st = state[b]
            fills = []

            def mk(t):
                def go():
                    tp = psA.tile([128, 512], BF16, tag="psA", name="vtp")
                    nc.tensor.transpose(
                        tp[:, 0:128], st["v"][:, t * 128 : (t + 1) * 128], ident
                    )
                    # one copy: [128, 2, 64] strided dst (skip ones columns)
                    dst = st["vS"][:, t, :].rearrange(
                        "p (g i) -> p g i", g=2, i=65
                    )[:, :, 0:64]
                    src = tp[:, 0:128].rearrange("p (g i) -> p g i", g=2, i=64)
                    nc.vector.tensor_copy(dst, src)
                    return 140
                return go

            for t in range(NKC):
                fills.append(mk(t))
            return fills

        def outproj_fillers(b, qt):
            """Output projection for q-tile qt of batch b: per token tile,
            2 matmuls + 2 DVE evacs + 1 DMA, split into 2 closures."""
            st = state[b]
            aTq = st[f"aT{qt}"]
            fills = []
            if b == B - 1 and qt == 3:
                # last q-tile: M=1024 matmuls through the (by now idle)
                # score-psum pool for a short serial tail
                def mk_tail(lt):
                    def go():
                        po = psS.tile([128, 1024], F32, tag="psS")
                        for g in range(2):
                            nc.tensor.matmul(
                                po[:, g * 512 : (g + 1) * 512],
                                aTq[:, lt * 128 : (lt + 1) * 128],
                                wo_sb[:, g * 512 : (g + 1) * 512],
                                start=True,
                                stop=True,
                                skip_group_check=True,
                            )
                        ob = pob.tile([128, 1024], BF16, tag="ob", name="obt")
                        if lt % 2 == 0:
                            nc.vector.tensor_copy(ob, po)
                        else:
                            nc.scalar.copy(ob, po)
                        t0 = b * S + qt * 512 + lt * 128
                        nc.gpsimd.dma_start(out[t0 : t0 + 128, :], ob)
                        return 432
                    return go

                for lt in range(4):
                    fills.append(mk_tail(lt))
                return fills
            for lt in range(4):
                ps_ref = {}

                def mk(lt, ps_ref, g):
                    def go():
                        if g == 0:
                            ps_ref["ob"] = pob.tile(
                                [128, 1024], BF16, tag="ob", name="ob"
                            )
                        po = psA.tile([128, 512], F32, tag="psA")
                        nc.tensor.matmul(
                            po,
                            aTq[:, lt * 128 : (lt + 1) * 128],
                            wo_sb[:, g * 512 : (g + 1) * 512],
                            start=True,
                            stop=True,
                            skip_group_check=True,
                        )
                        ob = ps_ref["ob"]
                        nc.vector.tensor_copy(
                            ob[:, g * 512 : (g + 1) * 512], po
                        )
                        if g == 1:
                            t0 = b * S + qt * 512 + lt * 128
                            nc.gpsimd.dma_start(out[t0 : t0 + 128, :], ob)
                        return 216
                    return go

                for g in range(2):
                    fills.append(mk(lt, ps_ref, g))
            return fills

        # ------------------------------------------------------------------
        # pipelined streams
        # ------------------------------------------------------------------
        def decode(s):
            return s // 64, (s % 64) // 16, s % 16  # b, qt, kc

        n_steps = B * 64

        def emit_scores(s):
            b, qt, kc = decode(s)
            st = state[b]
            qT, kT = st["q"], st["k"]
            q0, q1 = qt * 512, (qt + 1) * 512
            k0 = kc * 128
            sp = psS.tile([128, 1024], F32, tag="psS")
            nc.tensor.matmul(
                sp[:, 0:512], kT[0:64, k0 : k0 + 128], qT[0:64, q0:q1],
                start=True, stop=True, tile_position=(0, 0),
                skip_group_check=True,
            )
            nc.tensor.matmul(
                sp[:, 512:1024], kT[64:128, k0 : k0 + 128], qT[64:128, q0:q1],
                start=True, stop=True, tile_position=(64, 0),
                skip_group_check=True,
            )
            st.setdefault("sp", {})[s] = sp

        def emit_act(s):
            b, qt, kc = decode(s)
            st = state[b]
            sp = st["sp"].pop(s)
            pt = ppt.tile([128, 1024], BF16, tag="pt", name=f"pt_{s % 10}")
            nc.scalar.activation(pt, sp, EXP, scale=float(SCALE))
            st.setdefault("pt", {})[s] = pt

        def emit_av(s):
            b, qt, kc = decode(s)
            st = state[b]
            pt = st["pt"].pop(s)
            nc.tensor.matmul(
                u01[0:65, 0:512], st["vS"][:, kc, 0:65], pt[:, 0:512],
                start=(kc == 0), stop=(kc == NKC - 1),
                skip_group_check=True,
            )
            nc.tensor.matmul(
                u01[0:65, 512:1024], st["vS"][:, kc, 65:130], pt[:, 512:1024],
                start=(kc == 0), stop=(kc == NKC - 1),
                skip_group_check=True,
            )
            if kc == NKC - 1:
                emit_dchain(b, qt)

        def emit_dchain(b, qt):
            """Normalize q-tile qt of batch b out of u01 into aT{qt}.

            Emits only the two DVE reads of u01 inline (so u01 is free for
            the next q-tile after ~1.5us); the rest of the chain (transpose
            d, reciprocal, broadcast roundtrip, normalize-multiply) is
            deferred into a filler closure so the in-order PE queue never
            blocks on it."""
            st = state[b]
            # unnormalized U + denominator row out of PSUM in ONE DVE copy
            # (frees u01 for the next q-tile as fast as possible)
            usb = pd.tile([65, 1024], BF16, tag="usb", name="usb", bufs=3)
            nc.scalar.copy(usb, u01[0:65, :])
            dsb = usb[64:65, :]

            def chain():
                # transpose d into partition-parallel layout with 8 tiny
                # stationary-[1,128] matmuls against a [1,1] ones moving tile
                tcol = psA.tile([128, 512], F32, tag="psA", name="tcol")
                for j in range(8):
                    nc.tensor.matmul(
                        tcol[:, j : j + 1],
                        dsb[0:1, j * 128 : (j + 1) * 128],
                        ones1[64:65, :],
                        start=True,
                        stop=True,
                        skip_group_check=True,
                    )
                dinvT = pd.tile([128, 8], F32, tag="dinvT", name="dinvT")
                nc.vector.reciprocal(dinvT, tcol[:, 0:8])
                # transpose back via PE: [128, 8] -> [8, 128]
                t8 = psA.tile([128, 512], F32, tag="psA", name="t8")
                nc.tensor.transpose(t8[0:8, 0:128], dinvT, identf)
                # roundtrip through DRAM to broadcast across partitions
                t8sb = pd.tile([8, 128], BF16, tag="t8sb", name="t8sb")
                nc.vector.tensor_copy(t8sb, t8[0:8, 0:128])
                dscr = dsc.tile([1, 1024], BF16, tag="dscr", name="dscr")
                nc.sync.dma_start(
                    dscr[0:1, :].rearrange("a (j i) -> (a j) i", j=8, i=128),
                    t8sb,
                )
                bc = pbc.tile([64, 1024], BF16, tag="bc", name="bc")
                nc.sync.dma_start(bc, dscr.to_broadcast((64, 1024)))
                # normalize: aT[h*64:(h+1)*64, q] = U * (1/d)
                aTq = pa.tile([128, 512], BF16, tag="aT", name=f"aT_{b}_{qt}")
                st[f"aT{qt}"] = aTq
                nc.vector.tensor_mul(
                    aTq[0:64, :], usb[0:64, 0:512], bc[:, 0:512]
                )
                nc.vector.tensor_mul(
                    aTq[64:128, :], usb[0:64, 512:1024], bc[:, 512:1024]
                )
                for f in outproj_fillers(b, qt):
                    defer_q.append(f)
                return 700

            fill_q.appendleft(chain)

        # ------------------------------------------------------------------
        # prologue: batch 0 — x DMAs, k(qc0) and q(qt0) inline; remaining
        # k-groups are emitted inline just before the scores that need
        # them (emission order is what creates sync deps, so a consumer
        # must never be emitted before its producer).
        # ------------------------------------------------------------------
        for _part in range(4):
            push_x_dmas(0, _part)
        init_batch_tiles(0)
        b0_k = qkv_fillers(0, "k")
        b0_q = qkv_fillers(0, "q")
        for f in b0_k[:8]:
            f()
        for f in b0_q[:8]:
            f()
        vf = qkv_fillers(0, "v")
        vt = vtrans_fillers(0)
        for qc in range(4):
            for f in vf[qc * 8 : (qc + 1) * 8]:
                fill_q.append(f)
            for f in vt[qc * 4 : (qc + 1) * 4]:
                fill_q.append(f)
            for f in b0_q[8 * (qc + 1) : 8 * (qc + 2)]:
                fill_q.append(f)

        # ------------------------------------------------------------------
        # main loop: step-pairs (2 score steps, 2 lagged AV steps, fillers).
        # The AV stream is a gated cursor: the kc==0 AV of each q-tile may
        # only be emitted one pair after the previous q-tile's d-chain, so
        # the in-order PE queue never blocks on the u01 evacuation.
        # ------------------------------------------------------------------
        av_state = {"next": 0, "gate": -1, "pair": 0}

        def pump_avs(limit):
            n = 0
            while av_state["next"] <= limit and n < 3:
                a = av_state["next"]
                kk = a % 16
                if kk == 0 and a > 0 and av_state["pair"] <= av_state["gate"]:
                    break
                emit_av(a)
                if kk == NKC - 1:
                    av_state["gate"] = av_state["pair"]
                av_state["next"] += 1
                n += 1
            return 2 * n

        for b in range(B):
            if b + 1 < B:
                push_x_dmas(b + 1, 0)
                push_x_dmas(b + 1, 1)
                init_batch_tiles(b + 1)
                for f in qkv_fillers(b + 1, "k"):
                    fill_q.append(f)
                vf = qkv_fillers(b + 1, "v")
                vt = vtrans_fillers(b + 1)
                for qc in range(4):
                    for f in vf[qc * 8 : (qc + 1) * 8]:
                        fill_q.append(f)
                    for f in vt[qc * 4 : (qc + 1) * 4]:
                        fill_q.append(f)
                for f in qkv_fillers(b + 1, "q"):
                    fill_q.append(f)
            for local in range(0, 64, 2):
                s = b * 64 + local
                if b + 1 < B and local == 4:
                    push_x_dmas(b + 1, 2)
                    push_x_dmas(b + 1, 3)
                if b == 0 and local in (4, 8, 12):
                    for f in b0_k[8 * (local // 4) : 8 * (local // 4 + 1)]:
                        f()
                emit_scores(s)
                emit_act(s)
                emit_scores(s + 1)
                emit_act(s + 1)
                lag = 2 if (b == B - 1 and local >= 48) else LAG
                n_av = pump_avs(s + 1 - lag)
                for _ in range(2 if b == B - 1 else 1):
                    if defer_q:
                        fill_q.append(defer_q.popleft())
                # act cadence per pair is ~2294ns; keep emitted PE work just
                # below it (scores pair ~432ns, each AV ~216ns)
                budget = 2250 - 432 - 216 * n_av
                if b == 0 and local < 24:
                    budget += 700  # prologue catch-up
                pop_fillers_budget(budget)
                av_state["pair"] += 1

        # tail: drain remaining AV steps and fillers
        while av_state["next"] < n_steps:
            pump_avs(n_steps - 1)
            pop_fillers(4)
            av_state["pair"] += 1
        drain_fillers()
        while defer_q:
            fill_q.append(defer_q.popleft())
        drain_fillers()

    return nc


_NC_CACHE = {}


def _get_nc(with_bias=False):
    key = with_bias
    if key not in _NC_CACHE:
        _NC_CACHE[key] = build_attention_nc(with_bias)
    return _NC_CACHE[key]


def _run(inputs, Wq, bq, Wk, bk, Wv, bv, Wo, bo, trace=False, **spmd_kwargs):
    X2 = np.asarray(inputs, dtype=np.float32).reshape(T, D)
    xT = X2.T.astype(NP_BF16).reshape(KC, 128, T)
    with_bias = bool(
        np.any(np.asarray(bq)) or np.any(np.asarray(bk)) or np.any(np.asarray(bv))
    )

    in_maps = []
    for c in range(N_CORES):
        cs = slice(c * 128, (c + 1) * 128)
        in_maps.append(
            {
                "xT": xT,
                "wq": np.ascontiguousarray(
                    np.asarray(Wq[:, cs]).astype(NP_BF16).reshape(KC, 128, 128).transpose(1, 0, 2)
                ),
                "wk": np.ascontiguousarray(
                    np.asarray(Wk[:, cs]).astype(NP_BF16).reshape(KC, 128, 128).transpose(1, 0, 2)
                ),
                "wv": np.ascontiguousarray(
                    np.asarray(Wv[:, cs]).astype(NP_BF16).reshape(KC, 128, 128).transpose(1, 0, 2)
                ),
                "bq": np.asarray(bq[cs]).astype(NP_BF16),
                "bk": np.asarray(bk[cs]).astype(NP_BF16),
                "bv": np.asarray(bv[cs]).astype(NP_BF16),
                "wo": np.ascontiguousarray(Wo[cs, :]).astype(NP_BF16),
            }
        )

    if not with_bias:
        for m in in_maps:
            m.pop("bq"), m.pop("bk"), m.pop("bv")
    res = run_bass_kernel_spmd(
        _get_nc(with_bias), in_maps, list(range(N_CORES)), trace=trace, **spmd_kwargs
    )
    acc = res.results[0]["out"].astype(np.float32)
    for c in range(1, N_CORES):
        acc += res.results[c]["out"].astype(np.float32)
    acc += np.asarray(bo, dtype=np.float32)[None, :]
    return acc.reshape(B, S, D), res


def kernel(inputs, Wq, bq, Wk, bk, Wv, bv, Wo, bo):
    out, _ = _run(inputs, Wq, bq, Wk, bk, Wv, bv, Wo, bo)
    return out


# revision 36
# speedup vs baseline: 1.0806x; 1.0032x over previous
"""Multi-head self-attention (B=4, S=2048, D=1024, H=16) on 8 NeuronCores.

Sharding: tensor-parallel over heads. Core c owns heads {2c, 2c+1} = 128
columns of Wq/Wk/Wv and 128 rows of Wo. Each core computes Q^T/K^T/V for its
two heads over all tokens, runs attention for its 8 (batch, head) pairs, and
produces a partial output O_c = A_c @ Wo_c. The all-reduce over the 8
partials is done on the host during unsharding.

v2: software-pipelined at key-chunk granularity so the tensor engine never
stalls on the softmax exp (which would drop it from its 2.4 GHz ramped
p-state back to 1.2 GHz). The scalar engine runs only the exp activations
(the true bottleneck at ~285 us); QKV projections for the next batch,
V transposes, and the output projection of the previous q-tile are emitted
as tensor-engine fillers between score/AV matmuls. Softmax normalization is
restructured: denominators (from a ones-column in the V stationary) are
transposed into partition-parallel layout with tiny stationary-[1,128]
matmuls, reciprocated as [128,8] on the DVE (203ns vs 3.3us for [1,512]),
transposed back via the PE, and applied in a fused normalize+evacuate
tensor_tensor multiply.
"""
import os
import sys

for _p in ("/opt/trn_rl_repo", "/root/.axon_site/_ro/trn_rl_repo"):
    if os.path.isdir(_p) and _p not in sys.path:
        sys.path.append(_p)

from collections import deque
from contextlib import ExitStack

import numpy as np
import ml_dtypes

import concourse.bass as bass
import concourse.tile as tile
from concourse import mybir
from concourse.bass_utils import run_bass_kernel_spmd
from concourse.masks import make_identity

BF16 = mybir.dt.bfloat16
F32 = mybir.dt.float32
EXP = mybir.ActivationFunctionType.Exp
NP_BF16 = ml_dtypes.bfloat16

B, S, D = 4, 2048, 1024
H, HD = 16, 64
N_CORES = 8
T = B * S  # 8192 tokens
KC = D // 128  # 8 contraction chunks
NKC = S // 128  # 16 key chunks per batch
SCALE = 1.0 / np.sqrt(HD)
LAG = 5  # AV stream lags the score stream by this many kc-steps

# ---------------------------------------------------------------------------
# Tile patches: this walrus build rejects instructions with more than one
# sync wait ("Too many sync wait commands"), so split extra waits into
# preceding same-engine nops, and replace the kernel-tail drain's wait list
# with a chain of single-wait SP nops.
# ---------------------------------------------------------------------------
_MAX_WAITS = 1
_patched = False


def _install_tile_patches():
    global _patched
    if _patched:
        return
    _patched = True
    from concourse.vector_clock import ScopedClock, VectorClock

    orig_lower = tile.TileContext._lower_ordered_insts

    def split_inst_waits(self, ordered):
        for bb_name in list(ordered.keys()):
            insts = ordered[bb_name]
            new = []
            for inst in insts:
                si = inst.sync_info
                if si is not None and len(si.on_wait) > _MAX_WAITS:
                    waits = list(si.on_wait)
                    head, tail = waits[:-_MAX_WAITS], waits[-_MAX_WAITS:]
                    for w in head:
                        nop = mybir.InstNoOp(
                            name=f"ws-{self.nc.next_id()}",
                            engine=inst.engine,
                            bass_nofuse=True,
                        )
                        nop.sync_info = mybir.SyncInfo(on_wait=[w], on_update=[])
                        new.append(nop)
                    inst.sync_info = mybir.SyncInfo(
                        on_wait=tail, on_update=list(si.on_update)
                    )
                new.append(inst)
            ordered[bb_name] = new
        return orig_lower(self, ordered)

    def split_drain_and_barrier(self, tick_clock, wait_clock):
        gc = tick_clock.global_clock
        ticks = eval(repr(gc).replace("VectorClock", ""))
        procs = [(i, t) for i, t in enumerate(ticks) if t > 0]
        for i in range(0, len(procs), _MAX_WAITS):
            chunk = procs[i : i + _MAX_WAITS]
            nop = self.nc.sync.nop(nofuse=True, hint="drain_wait_split")
            pc = VectorClock()
            for proc, tick in chunk:
                pc.require_at_least(proc, tick)
            wait_clock.add_sem_waits(nop.ins, ScopedClock({None: pc}))
        drain_inst = self.nc.sync.drain()
        wait_clock.add_sem_waits(
            drain_inst.ins, ScopedClock({None: gc}), ScopedClock({None: gc.copy()})
        )
        self.nc.all_engine_barrier()
        assert self.sems is not None
        popped = self.nc._tile_sem_poison_stack.pop()
        assert popped is self._sem_poison
        self.nc.clear_and_free_semaphores(list(self.sems.allocated().values()))
        self.nc.all_engine_barrier()

    tile.TileContext._lower_ordered_insts = split_inst_waits
    tile.TileContext._drain_and_barrier = split_drain_and_barrier


# ---------------------------------------------------------------------------
# Device kernel
# ---------------------------------------------------------------------------
def build_attention_nc(with_bias=False):
    _install_tile_patches()
    nc = bass.Bass()

    xT = nc.declare_dram_parameter("xT", [KC, 128, T], BF16, isOutput=False)
    wq = nc.declare_dram_parameter("wq", [128, KC, 128], BF16, isOutput=False)
    wk = nc.declare_dram_parameter("wk", [128, KC, 128], BF16, isOutput=False)
    wv = nc.declare_dram_parameter("wv", [128, KC, 128], BF16, isOutput=False)
    if with_bias:
        bq = nc.declare_dram_parameter("bq", [128], BF16, isOutput=False)
        bk = nc.declare_dram_parameter("bk", [128], BF16, isOutput=False)
        bv = nc.declare_dram_parameter("bv", [128], BF16, isOutput=False)
    else:
        bq = bk = bv = None
    wo = nc.declare_dram_parameter("wo", [128, D], BF16, isOutput=False)
    out = nc.declare_dram_parameter("out", [T, D], BF16, isOutput=True)

    with tile.TileContext(nc) as tc, ExitStack() as ctx:
        singles = ctx.enter_context(tc.tile_pool(name="singles", bufs=1))
        px = ctx.enter_context(tc.tile_pool(name="px", bufs=16))
        pqk = ctx.enter_context(tc.tile_pool(name="pqk", bufs=2))
        pv = ctx.enter_context(tc.tile_pool(name="pv", bufs=2))
        ppt = ctx.enter_context(tc.tile_pool(name="ppt", bufs=10))
        pa = ctx.enter_context(tc.tile_pool(name="pa", bufs=4))
        pob = ctx.enter_context(tc.tile_pool(name="pob", bufs=3))
        pd = ctx.enter_context(tc.tile_pool(name="pd", bufs=2))
        pbc = ctx.enter_context(tc.tile_pool(name="pbc", bufs=3))
        dsc = ctx.enter_context(tc.tile_pool(name="dsc", bufs=4, space="DRAM"))
        psS = ctx.enter_context(tc.tile_pool(name="psS", bufs=2, space="PSUM"))
        psU = ctx.enter_context(tc.tile_pool(name="psU", bufs=1, space="PSUM"))
        psQ = ctx.enter_context(tc.tile_pool(name="psQ", bufs=1, space="PSUM"))
        psA = ctx.enter_context(tc.tile_pool(name="psA", bufs=1, space="PSUM"))

        # --- constants / weights, loaded once -----------------------------
        w_sb = {}
        b_sb = {}
        for name, wd, bd in (("k", wk, bk), ("q", wq, bq), ("v", wv, bv)):
            w_t = singles.tile([128, KC, 128], BF16, tag=f"w{name}")
            eng = nc.sync if name == "k" else nc.gpsimd
            eng.dma_start(w_t, wd[:, :, :])
            w_sb[name] = w_t
            if with_bias:
                b_t = singles.tile([1, 128], BF16, tag=f"b{name}")
                nc.sync.dma_start(b_t, bd[:][None, :])
                b_sb[name] = b_t
        wo_sb = singles.tile([128, D], BF16, tag="wo")
        nc.gpsimd.dma_start(wo_sb, wo[:, :])
        ones_row = singles.tile([1, 512], BF16, tag="ones_row")
        nc.vector.memset(ones_row, 1.0)
        ones1 = singles.tile([128, 1], BF16, tag="ones1")
        nc.vector.memset(ones1, 1.0)
        ident = singles.tile([128, 128], BF16, tag="ident")
        make_identity(nc, ident)
        identf = singles.tile([128, 128], F32, tag="identf")
        make_identity(nc, identf)

        # u01: single PSUM accumulator [128, 1024] f32; head0 in cols 0:512,
        # head1 in cols 512:1024; row 64 collects softmax denominators via
        # the ones column (index 64 resp. 129) in the vS stationary.
        u01 = psU.tile([128, 1024], F32, tag="u01")

        state = [dict() for _ in range(B)]

        # ------------------------------------------------------------------
        # filler machinery: small closures, each ~1-2 PE instructions (plus
        # attached DVE/DMA work). Popped between score/AV steps.
        # ------------------------------------------------------------------
        fill_q = deque()
        defer_q = deque()

        def pop_fillers_budget(budget_ns):
            # fillers return their PE cost (ns); pop until budget exhausted
            spent = 0
            while fill_q and spent < budget_ns:
                spent += fill_q.popleft()() or 216
            return spent

        def pop_fillers(n):
            for _ in range(n):
                if not fill_q:
                    return
                fill_q.popleft()()

        def drain_fillers():
            while fill_q:
                fill_q.popleft()()

        # --- QKV projection / V transpose fillers for batch b -------------
        def push_x_dmas(b, part=0):
            """part 0: allocate tiles + first token-quarter slices (needed
            first by the QKV fillers); parts 1..3: remaining quarters,
            staggered so the Sync queue is never occupied for long."""
            st = state[b]
            if part == 0:
                st["x"] = [
                    px.tile([128, S], BF16, tag="x", name=f"x_{b}_{kc}")
                    for kc in range(KC)
                ]
            lo, hi = part * 512, (part + 1) * 512
            eng = nc.sync if part < 2 else nc.gpsimd
            for kc in range(KC):
                eng.dma_start(
                    st["x"][kc][:, lo:hi],
                    xT[kc, :, b * S + lo : b * S + hi],
                )

        def init_batch_tiles(b):
            st = state[b]
            st["q"] = pqk.tile([128, S], BF16, tag="qT", name=f"qT_{b}")
            st["k"] = pqk.tile([128, S], BF16, tag="kT", name=f"kT_{b}")
            st["v"] = pv.tile([128, S], BF16, tag="vT", name=f"vT_{b}")
            # vS: [keys, key-chunk, 130]; cols 0:64 = V head0, col 64 = ones,
            # cols 65:129 = V head1, col 129 = ones.
            st["vS"] = pv.tile([128, NKC, 130], BF16, tag="vS", name=f"vS_{b}")
            nc.vector.memset(st["vS"][:, :, 64:65], 1.0)
            nc.vector.memset(st["vS"][:, :, 129:130], 1.0)

        def qkv_fillers(b, name):
            """Per (tensor, q-chunk): 8 accumulating matmuls + DVE evac,
            one matmul per filler closure to smooth PE load."""
            st = state[b]
            w_t = w_sb[name]
            fills = []
            for qc in range(4):
                ps_ref = {}

                def mk(qc, ps_ref, kc):
                    def go():
                        if kc == 0:
                            ps_ref["ps"] = psQ.tile(
                                [128, 512], F32, tag="psQ", name="qkv_ps"
                            )
                        ps = ps_ref["ps"]
                        nc.tensor.matmul(
                            ps,
                            w_t[:, kc, :],
                            st["x"][kc][:, qc * 512 : (qc + 1) * 512],
                            start=(kc == 0),
                            stop=(not with_bias and kc == KC - 1),
                            skip_group_check=True,
                        )
                        if kc == KC - 1:
                            if with_bias:
                                nc.tensor.matmul(
                                    ps, b_sb[name], ones_row,
                                    start=False, stop=True,
                                    skip_group_check=True,
                                )
                            nc.vector.tensor_copy(
                                st[name][:, qc * 512 : (qc + 1) * 512], ps
                            )
                        return 216
                    return go

                for kc in range(KC):
                    fills.append(mk(qc, ps_ref, kc))
            return fills

        def vtrans_fillers(b):
            """16 transposes of vT into token-major vS (+1 DVE copy each)."""
            st = state[b]
            fills = []

            def mk(t):
                def go():
                    tp = psA.tile([128, 512], BF16, tag="psA", name="vtp")
                    nc.tensor.transpose(
                        tp[:, 0:128], st["v"][:, t * 128 : (t + 1) * 128], ident
                    )
                    # one copy: [128, 2, 64] strided dst (skip ones columns)
                    dst = st["vS"][:, t, :].rearrange(
                        "p (g i) -> p g i", g=2, i=65
                    )[:, :, 0:64]
                    src = tp[:, 0:128].rearrange("p (g i) -> p g i", g=2, i=64)
                    nc.vector.tensor_copy(dst, src)
                    return 140
                return go

            for t in range(NKC):
                fills.append(mk(t))
            return fills

        def outproj_fillers(b, qt):
            """Output projection for q-tile qt of batch b: per token tile,
            2 matmuls + 2 DVE evacs + 1 DMA, split into 2 closures."""
            st = state[b]
            aTq = st[f"aT{qt}"]
            fills = []
            if b == B - 1 and qt == 3:
                # last q-tile: M=1024 matmuls through the (by now idle)
                # score-psum pool for a short serial tail
                def mk_tail(lt):
                    def go():
                        po = psS.tile([128, 1024], F32, tag="psS")
                        for g in range(2):
                            nc.tensor.matmul(
                                po[:, g * 512 : (g + 1) * 512],
                                aTq[:, lt * 128 : (lt + 1) * 128],
                                wo_sb[:, g * 512 : (g + 1) * 512],
                                start=True,
                                stop=True,
                                skip_group_check=True,
                            )
                        ob = pob.tile([128, 1024], BF16, tag="ob", name="obt")
                        if lt % 2 == 0:
                            nc.vector.tensor_copy(ob, po)
                        else:
                            nc.scalar.copy(ob, po)
                        t0 = b * S + qt * 512 + lt * 128
                        nc.gpsimd.dma_start(out[t0 : t0 + 128, :], ob)
                        return 432
                    return go

                for lt in range(4):
                    fills.append(mk_tail(lt))
                return fills
            for lt in range(4):
                ps_ref = {}

                def mk(lt, ps_ref, g):
                    def go():
                        if g == 0:
                            ps_ref["ob"] = pob.tile(
                                [128, 1024], BF16, tag="ob", name="ob"
                            )
                        po = psA.tile([128, 512], F32, tag="psA")
                        nc.tensor.matmul(
                            po,
                            aTq[:, lt * 128 : (lt + 1) * 128],
                            wo_sb[:, g * 512 : (g + 1) * 512],
                            start=True,
                            stop=True,
                            skip_group_check=True,
                        )
                        ob = ps_ref["ob"]
                        nc.vector.tensor_copy(
                            ob[:, g * 512 : (g + 1) * 512], po
                        )
                        if g == 1:
                            t0 = b * S + qt * 512 + lt * 128
                            nc.gpsimd.dma_start(out[t0 : t0 + 128, :], ob)
                        return 216
                    return go

                for g in range(2):
                    fills.append(mk(lt, ps_ref, g))
            return fills

        # ------------------------------------------------------------------
        # pipelined streams
        # ------------------------------------------------------------------
        def decode(s):
            return s // 64, (s % 64) // 16, s % 16  # b, qt, kc

        n_steps = B * 64

        def emit_scores(s):
            b, qt, kc = decode(s)
            st = state[b]
            qT, kT = st["q"], st["k"]
            q0, q1 = qt * 512, (qt + 1) * 512
            k0 = kc * 128
            sp = psS.tile([128, 1024], F32, tag="psS")
            nc.tensor.matmul(
                sp[:, 0:512], kT[0:64, k0 : k0 + 128], qT[0:64, q0:q1],
                start=True, stop=True, tile_position=(0, 0),
                skip_group_check=True,
            )
            nc.tensor.matmul(
                sp[:, 512:1024], kT[64:128, k0 : k0 + 128], qT[64:128, q0:q1],
                start=True, stop=True, tile_position=(64, 0),
                skip_group_check=True,
            )
            st.setdefault("sp", {})[s] = sp

        def emit_act(s):
            b, qt, kc = decode(s)
            st = state[b]
            sp = st["sp"].pop(s)
            pt = ppt.tile([128, 1024], BF16, tag="pt", name=f"pt_{s % 10}")
            nc.scalar.activation(pt, sp, EXP, scale=float(SCALE))
            st.setdefault("pt", {})[s] = pt

        def emit_av(s):
            b, qt, kc = decode(s)
            st = state[b]
            pt = st["pt"].pop(s)
            nc.tensor.matmul(
                u01[0:65, 0:512], st["vS"][:, kc, 0:65], pt[:, 0:512],
                start=(kc == 0), stop=(kc == NKC - 1),
                skip_group_check=True,
            )
            nc.tensor.matmul(
                u01[0:65, 512:1024], st["vS"][:, kc, 65:130], pt[:, 512:1024],
                start=(kc == 0), stop=(kc == NKC - 1),
                skip_group_check=True,
            )
            if kc == NKC - 1:
                emit_dchain(b, qt)

        def emit_dchain(b, qt):
            """Normalize q-tile qt of batch b out of u01 into aT{qt}.

            Emits only the two DVE reads of u01 inline (so u01 is free for
            the next q-tile after ~1.5us); the rest of the chain (transpose
            d, reciprocal, broadcast roundtrip, normalize-multiply) is
            deferred into a filler closure so the in-order PE queue never
            blocks on it."""
            st = state[b]
            # unnormalized U + denominator row out of PSUM in ONE DVE copy
            # (frees u01 for the next q-tile as fast as possible)
            usb = pd.tile([65, 1024], BF16, tag="usb", name="usb", bufs=3)
            nc.scalar.copy(usb, u01[0:65, :])
            dsb = usb[64:65, :]

            def chain():
                # transpose d into partition-parallel layout with 8 tiny
                # stationary-[1,128] matmuls against a [1,1] ones moving tile
                tcol = psA.tile([128, 512], F32, tag="psA", name="tcol")
                for j in range(8):
                    nc.tensor.matmul(
                        tcol[:, j : j + 1],
                        dsb[0:1, j * 128 : (j + 1) * 128],
                        ones1[64:65, :],
                        start=True,
                        stop=True,
                        skip_group_check=True,
                    )
                dinvT = pd.tile([128, 8], F32, tag="dinvT", name="dinvT")
                nc.vector.reciprocal(dinvT, tcol[:, 0:8])
                # transpose back via PE: [128, 8] -> [8, 128]
                t8 = psA.tile([128, 512], F32, tag="psA", name="t8")
                nc.tensor.transpose(t8[0:8, 0:128], dinvT, identf)
                # roundtrip through DRAM to broadcast across partitions
                t8sb = pd.tile([8, 128], BF16, tag="t8sb", name="t8sb")
                nc.vector.tensor_copy(t8sb, t8[0:8, 0:128])
                dscr = dsc.tile([1, 1024], BF16, tag="dscr", name="dscr")
                nc.sync.dma_start(
                    dscr[0:1, :].rearrange("a (j i) -> (a j) i", j=8, i=128),
                    t8sb,
                )
                bc = pbc.tile([64, 1024], BF16, tag="bc", name="bc")
                nc.sync.dma_start(bc, dscr.to_broadcast((64, 1024)))
                # normalize: aT[h*64:(h+1)*64, q] = U * (1/d)
                aTq = pa.tile([128, 512], BF16, tag="aT", name=f"aT_{b}_{qt}")
                st[f"aT{qt}"] = aTq
                nc.vector.tensor_mul(
                    aTq[0:64, :], usb[0:64, 0:512], bc[:, 0:512]
                )
                nc.vector.tensor_mul(
                    aTq[64:128, :], usb[0:64, 512:1024], bc[:, 512:1024]
                )
                for f in outproj_fillers(b, qt):
                    defer_q.append(f)
                return 700

            fill_q.appendleft(chain)

        # ------------------------------------------------------------------
        # prologue: batch 0 — x DMAs, k(qc0) and q(qt0) inline; remaining
        # k-groups are emitted inline just before the scores that need
        # them (emission order is what creates sync deps, so a consumer
        # must never be emitted before its producer).
        # ------------------------------------------------------------------
        for _part in range(4):
            push_x_dmas(0, _part)
        init_batch_tiles(0)
        b0_k = qkv_fillers(0, "k")
        b0_q = qkv_fillers(0, "q")
        for f in b0_k[:8]:
            f()
        for f in b0_q[:8]:
            f()
        vf = qkv_fillers(0, "v")
        vt = vtrans_fillers(0)
        for qc in range(4):
            for f in vf[qc * 8 : (qc + 1) * 8]:
                fill_q.append(f)
            for f in vt[qc * 4 : (qc + 1) * 4]:
                fill_q.append(f)
            for f in b0_q[8 * (qc + 1) : 8 * (qc + 2)]:
                fill_q.append(f)

        # ------------------------------------------------------------------
        # main loop: step-pairs (2 score steps, 2 lagged AV steps, fillers).
        # The AV stream is a gated cursor: the kc==0 AV of each q-tile may
        # only be emitted one pair after the previous q-tile's d-chain, so
        # the in-order PE queue never blocks on the u01 evacuation.
        # ------------------------------------------------------------------
        av_state = {"next": 0, "gate": -1, "pair": 0}

        def pump_avs(limit):
            n = 0
            while av_state["next"] <= limit and n < 3:
                a = av_state["next"]
                kk = a % 16
                if kk == 0 and a > 0 and av_state["pair"] <= av_state["gate"]:
                    break
                emit_av(a)
                if kk == NKC - 1:
                    av_state["gate"] = av_state["pair"]
                av_state["next"] += 1
                n += 1
            return 2 * n

        for b in range(B):
            if b + 1 < B:
                push_x_dmas(b + 1, 0)
                push_x_dmas(b + 1, 1)
                init_batch_tiles(b + 1)
                for f in qkv_fillers(b + 1, "k"):
                    fill_q.append(f)
                qf = qkv_fillers(b + 1, "q")
                for f in qf[:8]:
                    fill_q.append(f)
                vf = qkv_fillers(b + 1, "v")
                vt = vtrans_fillers(b + 1)
                for qc in range(4):
                    for f in vf[qc * 8 : (qc + 1) * 8]:
                        fill_q.append(f)
                    for f in vt[qc * 4 : (qc + 1) * 4]:
                        fill_q.append(f)
                for f in qf[8:]:
                    fill_q.append(f)
            for local in range(0, 64, 2):
                s = b * 64 + local
                if b + 1 < B and local == 4:
                    push_x_dmas(b + 1, 2)
                    push_x_dmas(b + 1, 3)
                if b == 0 and local in (4, 8, 12):
                    for f in b0_k[8 * (local // 4) : 8 * (local // 4 + 1)]:
                        f()
                emit_scores(s)
                emit_act(s)
                emit_scores(s + 1)
                emit_act(s + 1)
                lag = 2 if (b == B - 1 and local >= 48) else LAG
                n_av = pump_avs(s + 1 - lag)
                for _ in range(2 if b == B - 1 else 1):
                    if defer_q:
                        fill_q.append(defer_q.popleft())
                # act cadence per pair is ~2294ns; keep emitted PE work just
                # below it (scores pair ~432ns, each AV ~216ns)
                budget = 2250 - 432 - 216 * n_av
                if b == 0 and local < 24:
                    budget += 700  # prologue catch-up
                if b == B - 1:
                    budget += 800  # no QKV fillers; drain outproj backlog
                pop_fillers_budget(budget)
                av_state["pair"] += 1

        # tail: drain remaining AV steps and fillers
        while av_state["next"] < n_steps:
            pump_avs(n_steps - 1)
            pop_fillers(8)
            av_state["pair"] += 1
        drain_fillers()
        while defer_q:
            fill_q.append(defer_q.popleft())
        drain_fillers()

    return nc


_NC_CACHE = {}


def _get_nc(with_bias=False):
    key = with_bias
    if key not in _NC_CACHE:
        _NC_CACHE[key] = build_attention_nc(with_bias)
    return _NC_CACHE[key]


def _run(inputs, Wq, bq, Wk, bk, Wv, bv, Wo, bo, trace=False, **spmd_kwargs):
    X2 = np.asarray(inputs, dtype=np.float32).reshape(T, D)
    xT = X2.T.astype(NP_BF16).reshape(KC, 128, T)
    with_bias = bool(
        np.any(np.asarray(bq)) or np.any(np.asarray(bk)) or np.any(np.asarray(bv))
    )

    in_maps = []
    for c in range(N_CORES):
        cs = slice(c * 128, (c + 1) * 128)
        in_maps.append(
            {
                "xT": xT,
                "wq": np.ascontiguousarray(
                    np.asarray(Wq[:, cs]).astype(NP_BF16).reshape(KC, 128, 128).transpose(1, 0, 2)
                ),
                "wk": np.ascontiguousarray(
                    np.asarray(Wk[:, cs]).astype(NP_BF16).reshape(KC, 128, 128).transpose(1, 0, 2)
                ),
                "wv": np.ascontiguousarray(
                    np.asarray(Wv[:, cs]).astype(NP_BF16).reshape(KC, 128, 128).transpose(1, 0, 2)
                ),
                "bq": np.asarray(bq[cs]).astype(NP_BF16),
                "bk": np.asarray(bk[cs]).astype(NP_BF16),
                "bv": np.asarray(bv[cs]).astype(NP_BF16),
                "wo": np.ascontiguousarray(Wo[cs, :]).astype(NP_BF16),
            }
        )

    if not with_bias:
        for m in in_maps:
            m.pop("bq"), m.pop("bk"), m.pop("bv")
    res = run_bass_kernel_spmd(
        _get_nc(with_bias), in_maps, list(range(N_CORES)), trace=trace, **spmd_kwargs
    )
    acc = res.results[0]["out"].astype(np.float32)
    for c in range(1, N_CORES):
        acc += res.results[c]["out"].astype(np.float32)
    acc += np.asarray(bo, dtype=np.float32)[None, :]
    return acc.reshape(B, S, D), res


def kernel(inputs, Wq, bq, Wk, bk, Wv, bv, Wo, bo):
    out, _ = _run(inputs, Wq, bq, Wk, bk, Wv, bv, Wo, bo)
    return out
